# revision 1
# baseline (speedup 1.0000x reference)
"""Trainium2 Bass kernel for the LayerNorm-RNN attention variant.

Math (per batch element b, reference semantics):
    u_t   = (x_t @ W_e2s + b_e2s) @ Bm                      # injected input
    y_t   = s_{t-1} @ A + u_t
    s_t   = LN(y_t) * gamma + beta                          # LayerNorm over S
    out_t = (s_t @ C) @ W_s2o + b_s2o

Host-side folds (exact linear algebra, done once in fp32/fp64):
    W_u  = W_e2s @ Bm                  u'_t = x_t @ W_u + (b_e2s @ Bm + beta @ A)
    G    = diag(gamma) @ A  (bf16)     whitened state t_t: s_t = t_t*gamma + beta
    W_o  = (diag(gamma) @ C) @ W_s2o   b_out = beta @ C @ W_s2o + b_s2o
    grow = colsums(G)                  (mean-correction direction)

Scan recurrence implemented with DELAYED NORMALIZATION so the serial
cross-engine tail per step is a single fused DVE op:
    state z_k = y_k (pre-norm), scalars rr_k = rsqrt(var_k+eps), mneg_k = -mu_k*rr_k
    W_k     = z_k @ G                    (PE, 16 [128,128] bf16 tiles)
    z_{k+1} = rr_k * W_k + u'_{k+1} + grow * mneg_k          (one STT op)
    t_k     = rr_k * z_k + mneg_k       -> states buffer (off critical path)

Sharding: data-parallel over batch, 1 batch element per NeuronCore (8 cores).
Layouts on chip are column-form: S=512 lives as [128 partitions x 4 free].
"""

import sys
import os
from contextlib import ExitStack

import numpy as np

for _p in ("/opt/trn_rl_repo",):
    if _p not in sys.path and os.path.isdir(_p):
        sys.path.insert(0, _p)

B, T, E, S = 8, 2048, 1024, 512
LN_EPS = 1e-5
NCORES = 8
UNROLL = 256

_CACHE = {}


def build(t_len=T, unroll=UNROLL, debug_dumps=False, phase="full", lite=0,
          warm_mms=0):
    """Build the single-core Bass program (SPMD across 8 cores)."""
    import concourse.bass as bass
    import concourse.bacc as bacc
    from concourse import mybir
    from concourse.tile import TileContext
    from concourse.tile_rust import add_dep_helper

    f32 = mybir.dt.float32
    bf16 = mybir.dt.bfloat16
    AF = mybir.ActivationFunctionType
    ALU = mybir.AluOpType
    ds = bass.ds

    n_iters = t_len // unroll
    assert n_iters * unroll == t_len and unroll % 2 == 0
    n_tc = t_len // 512 if t_len >= 512 else 1   # pre-pass t-chunks
    tcw = min(512, t_len)                        # pre-pass chunk width
    pcw = min(128, t_len)                        # post-pass chunk width
    n_pc = (t_len + pcw - 1) // pcw              # post-pass t-chunks

    nc = bacc.Bacc(trn_type="TRN2")

    xt = nc.dram_tensor("xt", [E, t_len], f32, kind="ExternalInput")
    wu = nc.dram_tensor("wu", [8, 4, 128, 128], f32, kind="ExternalInput")
    gt = nc.dram_tensor("gt", [4, 4, 128, 128], bf16, kind="ExternalInput")
    wo = nc.dram_tensor("wo", [S, E], bf16, kind="ExternalInput")
    growd = nc.dram_tensor("growc", [128, 4], f32, kind="ExternalInput")
    bud = nc.dram_tensor("buc", [128, 4], f32, kind="ExternalInput")
    bod = nc.dram_tensor("bo", [1, E], f32, kind="ExternalInput")
    cnegd = nc.dram_tensor("cneg", [128, 4], f32, kind="ExternalInput")
    onesd = nc.dram_tensor("ones", [128, 128], bf16, kind="ExternalInput")
    y = nc.dram_tensor("y", [t_len, E], f32, kind="ExternalOutput")
    if debug_dumps:
        dbg_u = nc.dram_tensor("dbg_u", [128, (t_len + 1) * 4], f32, kind="ExternalOutput")
        dbg_st = nc.dram_tensor("dbg_st", [128, t_len * 4], f32, kind="ExternalOutput")
        dbg_bo = nc.dram_tensor("dbg_bo", [128, E], f32, kind="ExternalOutput")
        dbg_ps = nc.dram_tensor("dbg_ps", [128, 512], f32, kind="ExternalOutput")

    with ExitStack() as ctx:
        tc = ctx.enter_context(TileContext(nc))
        singles = ctx.enter_context(tc.tile_pool(name="singles", bufs=1))
        xpool = ctx.enter_context(tc.tile_pool(name="xpool", bufs=16))
        psum_big = ctx.enter_context(tc.tile_pool(name="psum_big", bufs=2, space="PSUM"))
        psum_w = ctx.enter_context(tc.tile_pool(name="psum_w", bufs=3, space="PSUM"))
        psum_ab = ctx.enter_context(tc.tile_pool(name="psum_ab", bufs=3, space="PSUM"))
        temps = ctx.enter_context(tc.tile_pool(name="temps", bufs=4))
        opool = ctx.enter_context(tc.tile_pool(name="opool", bufs=2))

        # ---- resident weights / constants ----
        wu_sb = singles.tile([128, 8, 4, 128], f32)
        nc.sync.dma_start(out=wu_sb, in_=wu.rearrange("k m p q -> p k m q"))
        gt_sb = singles.tile([128, 4, 4, 128], bf16)
        nc.sync.dma_start(out=gt_sb, in_=gt.rearrange("k m p q -> p k m q"))
        wo_sb = singles.tile([128, 4, E], bf16)
        nc.sync.dma_start(out=wo_sb, in_=wo.rearrange("(k p) e -> p k e", p=128))
        grow_sb = singles.tile([128, 4], f32)
        nc.sync.dma_start(out=grow_sb, in_=growd[:])
        bu_sb = singles.tile([128, 4], f32)
        nc.sync.dma_start(out=bu_sb, in_=bud[:])
        ones_sb = singles.tile([128, 128], bf16)
        nc.sync.dma_start(out=ones_sb, in_=onesd[:])
        bo_ap = bod[:]
        bo_sb = singles.tile([128, E], f32)
        nc.sync.dma_start(
            out=bo_sb,
            in_=bass.AP(tensor=bo_ap.tensor, offset=bo_ap.offset, ap=[[0, 128], [1, E]]),
        )
        cneg_sb = singles.tile([128, 4], f32)
        nc.sync.dma_start(out=cneg_sb, in_=cnegd[:])
        eps_sb = singles.tile([128, 1], f32)
        nc.vector.memset(eps_sb, LN_EPS)

        u_col = singles.tile([128, (t_len + 1) * 4], f32)
        states = singles.tile([128, t_len * 4], bf16)
        u_view = u_col.rearrange("p (t f) -> p t f", f=4)
        st_view = states.rearrange("p (t f) -> p t f", f=4)
        nc.vector.memset(u_col[:, t_len * 4:(t_len + 1) * 4], 0.0)

        # ---- pre-pass: u'_col[s, t] = (x @ W_u + b_u).T in column form ----
        for c in range(n_tc):
            xts = []
            for e in range(8):
                xt_t = xpool.tile([128, tcw], f32, tag="xt")
                nc.sync.dma_start(
                    out=xt_t, in_=xt[e * 128:(e + 1) * 128, c * tcw:(c + 1) * tcw]
                )
                xts.append(xt_t)
            for m in range(4):
                ps = psum_big.tile([128, tcw], f32)
                for k in range(8):
                    nc.tensor.matmul(
                        ps, wu_sb[:, k, m, :], xts[k], start=(k == 0), stop=(k == 7)
                    )
                nc.scalar.activation(
                    out=u_view[:, c * tcw:(c + 1) * tcw, m],
                    in_=ps,
                    func=AF.Identity,
                    bias=bu_sb[:, m:m + 1],
                    scale=1.0,
                )

        # ---- scan ----
        if phase == "pre":
            ob0 = opool.tile([128, E], f32)
            nc.vector.tensor_copy(ob0[:, 0:4], u_col[:, 0:4])
            nc.sync.dma_start(out=y[0:pcw, :], in_=ob0[:pcw, :])
        do_scan = phase in ("scan", "full")
        z_a = singles.tile([128, 8], bf16)
        z_b = singles.tile([128, 8], bf16)
        rb_a = singles.tile([128, 2], f32)
        rb_b = singles.tile([128, 2], f32)
        gm_a = singles.tile([128, 4], f32)
        gm_b = singles.tile([128, 4], f32)
        uq_a = singles.tile([128, 4], f32)
        uq_b = singles.tile([128, 4], f32)
        # staging tiles: one dynamic DMA per loop iteration instead of one
        # register-offset AP per step (engines run out of registers past ~6)
        u_stg = singles.tile([128, (unroll + 1) * 4], f32)
        st_stg = singles.tile([128, unroll * 4], bf16)

        # prologue: z_0 = u'_0 - beta@A (state at t=-1 is exactly zero, so the
        # beta-fold baked into b_u must be removed for step 0)
        if do_scan:
            if lite in (11, 12):
                nc.vector.tensor_add(z_a[:, 0:4], u_col[:, 0:4], cneg_sb)
            else:
                nc.vector.scalar_tensor_tensor(
                    out=z_a[:, 0:4],
                    in0=u_col[:, 0:4],
                    scalar=1.0,
                    in1=cneg_sb,
                    op0=ALU.mult,
                    op1=ALU.add,
                )

        def scan_step(kexpr, jj):
            even = jj % 2 == 0
            cur_z, nxt_z = (z_a, z_b) if even else (z_b, z_a)
            rb = rb_a if even else rb_b
            gm = gm_a if even else gm_b
            uq = uq_a if even else uq_b
            zc = cur_z[:, 0:4]
            zsq = cur_z[:, 4:8]
            # zsq = z*z (cols 4:8 of the z tile)
            nc.vector.tensor_mul(zsq, zc, zc)
            # W = z @ G. Issue the first m-column, then the stats matmul (so
            # it isn't stuck behind the whole W-block in PE program order),
            # then the remaining three m-columns.
            wp = psum_w.tile([128, 4], f32)
            for kk in range(4):
                nc.tensor.matmul(
                    wp[:, 0:1], gt_sb[:, kk, 0, :], zc[:, kk:kk + 1],
                    start=(kk == 0), stop=(kk == 3),
                )
            # per-column partition sums of [z|zsq], broadcast to all partitions
            ab8 = psum_ab.tile([128, 8], f32)
            nc.tensor.matmul(ab8, ones_sb, cur_z, start=True, stop=True,
                             skip_group_check=True)
            for m in range(1, 4):
                for kk in range(4):
                    nc.tensor.matmul(
                        wp[:, m:m + 1], gt_sb[:, kk, m, :], zc[:, kk:kk + 1],
                        start=(kk == 0), stop=(kk == 3),
                    )
            # ab_sb = [mu | E[z^2]] (free-dim reduce of the column sums)
            ab_sb = temps.tile([128, 2], f32, tag="absb")
            nc.vector.tensor_reduce(
                out=ab_sb, in_=ab8.rearrange("p (a b) -> p a b", b=4),
                axis=mybir.AxisListType.X, op=ALU.add,
            )
            # nv = mu^2 - E[z^2] = -var
            nv = temps.tile([128, 1], f32, tag="nv")
            nc.vector.tensor_scalar(
                out=nv, in0=ab_sb[:, 0:1], scalar1=ab_sb[:, 0:1],
                scalar2=ab_sb[:, 1:2], op0=ALU.mult, op1=ALU.subtract,
            )
            # rr = 1/sqrt(var + eps) in one ACT op
            nc.scalar.activation(
                out=rb[:, 0:1], in_=nv, func=AF.Abs_reciprocal_sqrt,
                bias=eps_sb, scale=-1.0,
            )
            # gmu = -mu*grow (DVE, runs while ACT computes rr)
            nc.vector.tensor_scalar(
                out=gm, in0=grow_sb, scalar1=ab_sb[:, 0:1], scalar2=-1.0,
                op0=ALU.mult, op1=ALU.mult,
            )
            # mneg = -mu*rr on GPSIMD (only needed by the states write on ACT)
            nc.gpsimd.tensor_scalar(
                out=rb[:, 1:2], in0=ab_sb[:, 0:1], scalar1=rb[:, 0:1],
                scalar2=-1.0, op0=ALU.mult, op1=ALU.mult,
            )
            # states[k] = rr*z + mneg  (whitened state; ACT, off the DVE path)
            nc.scalar.activation(
                out=st_stg[:, jj * 4:(jj + 1) * 4], in_=zc, func=AF.Identity,
                scale=rb[:, 0:1], bias=rb[:, 1:2],
            )
            # h = W + gmu (PSUM+SBUF; runs while ACT computes rr — off path)
            nc.vector.tensor_add(uq, wp, gm)
            # serial tail: z_{k+1} = rr*(W+gmu) + u'[k+1]; red[:,0] = sum(z)
            un = u_stg[:, (jj + 1) * 4:(jj + 2) * 4]
            nc.vector.scalar_tensor_tensor(
                out=nxt_z[:, 0:4], in0=uq, scalar=rb[:, 0:1], in1=un,
                op0=ALU.mult, op1=ALU.add,
            )

        ucw = unroll * 4
        if do_scan:
          with tc.For_i(0, n_iters, 1, hint_engines=(
                  mybir.EngineType.PE, mybir.EngineType.DVE,
                  mybir.EngineType.Activation)) as iv:
            # stage u'[k] for k in [iv*unroll .. iv*unroll+unroll] (shifted +1
            # step: p2 of step jj reads u'[iv*unroll+jj+1])
            if lite == 0 or (4 <= lite <= 9):
                nc.gpsimd.dma_start(out=u_stg, in_=u_col[:, ds(iv * ucw, ucw + 4)])
            for jj in range(unroll):
                scan_step(None, jj)
            # flush whitened states for this chunk
            if lite == 0 or (4 <= lite <= 9):
                nc.gpsimd.dma_start(out=states[:, ds(iv * ucw, ucw)], in_=st_stg)

        if debug_dumps:
            nc.sync.dma_start(out=dbg_u[:], in_=u_col)
            nc.sync.dma_start(out=dbg_st[:], in_=states)
            nc.sync.dma_start(out=dbg_bo[:], in_=bo_sb)

        # ---- post-pass: out = states @ W_o + b_out ----
        if phase == "scan" and (lite == 0 or (4 <= lite <= 9)):
            ob0 = opool.tile([128, E], f32)
            nc.vector.tensor_copy(ob0[:, 0:4], states[:, 0:4])
            nc.sync.dma_start(out=y[0:pcw, :], in_=ob0[:pcw, :])
        for t_i in range(n_pc if phase == "full" else 0):
            ob = opool.tile([128, E], f32)
            for ec in range(2):
                ps = psum_big.tile([128, 512], f32)
                for kk in range(4):
                    nc.tensor.matmul(
                        ps[:pcw, :],
                        st_view[:, t_i * pcw:(t_i + 1) * pcw, kk],
                        wo_sb[:, kk, ec * 512:(ec + 1) * 512],
                        start=(kk == 0),
                        stop=(kk == 3),
                    )
                nc.vector.tensor_add(
                    ob[:pcw, ec * 512:(ec + 1) * 512], ps[:pcw, :],
                    bo_sb[:pcw, ec * 512:(ec + 1) * 512]
                )
                if debug_dumps and t_i == 0 and ec == 0:
                    dbg_ps_sb = opool.tile([128, 512], f32, tag="dbgps")
                    nc.vector.tensor_copy(dbg_ps_sb, ps)
                    nc.sync.dma_start(out=dbg_ps[:], in_=dbg_ps_sb)
            nc.sync.dma_start(out=y[t_i * pcw:(t_i + 1) * pcw, :], in_=ob[:pcw, :])

    nc.compile()
    return nc


def host_prep(inputs, t_len=T):
    """Fold parameters on the host; returns (shared dict, per-core xt list)."""
    from ml_dtypes import bfloat16

    et = np.asarray(inputs["embedded_tokens"], np.float32)
    W_e2s = np.asarray(inputs["W_e2s"], np.float64)
    b_e2s = np.asarray(inputs["b_e2s"], np.float64)
    A = np.asarray(inputs["A"], np.float64)
    Bm = np.asarray(inputs["Bm"], np.float64)
    C = np.asarray(inputs["C"], np.float64)
    gamma = np.asarray(inputs["ln_gamma"], np.float64)
    beta = np.asarray(inputs["ln_beta"], np.float64)
    W_s2o = np.asarray(inputs["W_s2o"], np.float64)
    b_s2o = np.asarray(inputs["b_s2o"], np.float64)

    W_u = (W_e2s @ Bm).astype(np.float32)                      # [E, S]
    b_u = (b_e2s @ Bm + beta @ A).astype(np.float32)           # [S]
    G = (gamma[:, None] * A).astype(np.float32)                # [S, S]
    Gb = G.astype(bfloat16)
    grow = Gb.astype(np.float32).sum(axis=0).astype(np.float32)  # [S] colsums
    W_o = ((gamma[:, None] * C) @ W_s2o).astype(np.float32)    # [S, E]
    b_out = (beta @ C @ W_s2o + b_s2o).astype(np.float32)      # [E]

    wu_tiles = np.ascontiguousarray(
        W_u.reshape(8, 128, 4, 128).transpose(0, 2, 1, 3)
    )  # [k, m, 128, 128]
    gt_tiles = np.ascontiguousarray(
        Gb.reshape(4, 128, 4, 128).transpose(0, 2, 1, 3)
    )  # [k, m, 128, 128] bf16

    shared = {
        "wu": wu_tiles.astype(np.float32),
        "gt": gt_tiles,
        "wo": np.ascontiguousarray(W_o.astype(bfloat16)),
        "growc": np.ascontiguousarray(grow.reshape(4, 128).T),
        "buc": np.ascontiguousarray(b_u.reshape(4, 128).T),
        "bo": np.ascontiguousarray(b_out.reshape(1, E)),
        "cneg": np.ascontiguousarray(
            (-(beta @ A)).astype(np.float32).reshape(4, 128).T
        ),
        "ones": np.full((128, 128), 1.0 / S, bfloat16),
    }
    xts = [
        np.ascontiguousarray(et[b, :t_len, :].T.astype(np.float32))
        for b in range(et.shape[0])
    ]
    return shared, xts


def kernel(**inputs):
    key = ("nc", T, UNROLL)
    if key not in _CACHE:
        _CACHE[key] = build(T, UNROLL)
    nc = _CACHE[key]

    from concourse.bass_utils import run_bass_kernel_spmd

    shared, xts = host_prep(inputs)
    in_maps = [dict(shared, xt=xts[b]) for b in range(B)]
    res = run_bass_kernel_spmd(nc, in_maps, core_ids=list(range(NCORES)))
    out = np.stack([np.asarray(r["y"], np.float32) for r in res.results], axis=0)
    return out



# revision 3
# speedup vs baseline: 1.5353x; 1.5353x over previous
"""Trainium2 Bass kernel for the LayerNorm-RNN attention variant.

Math (per batch element b, reference semantics):
    u_t   = (x_t @ W_e2s + b_e2s) @ Bm
    y_t   = s_{t-1} @ A + u_t
    s_t   = LN(y_t) * gamma + beta
    out_t = (s_t @ C) @ W_s2o + b_s2o

Key reformulation (all folds exact, done host-side in fp64):
  1. Mean-free weights: G = diag(gamma) @ A,  Gt = G - (G@1/S) 1^T has zero
     row-sums, so W = zc @ Gt is exactly zero-mean for any zc. Tracking the
     CENTERED pre-norm state zc kills the per-step mean/bias bookkeeping:
         zc_{t+1} = rr_t * (zc_t @ Gt) + uc_{t+1},   rr_t = rsqrt(|zc_t|^2/S + eps)
     with uc = centered input injection (centering matrix folded into W_u).
  2. Orthogonal Schur basis: Gt = Q T Q^T (real Schur, 2x2 blocks nudged off
     the 128-boundaries with dtrexc). w = zc @ Q keeps |w| = |zc| (stats
     unchanged) while T is block-upper-triangular: the per-step matvec needs
     only 10 of 16 [128,128] tiles.
  3. Whitened states tw_t = rr_t * w_t are accumulated and folded through
     W_O = Q^T diag(gamma) C W_s2o in a bulk post-pass.

Per-step engine schedule (serial ring kept minimal):
    DVE:  wsq = w*w
    PE :  3 early T tiles | stats: Sum_p wsq via 4 accumulating matmuls with
          an all-ones stationary (broadcast to all partitions) | 7 late tiles
    ACT:  rr = rsqrt(sum/S + eps)  directly from PSUM
    DVE:  w' = rr * W + uc_next    (single scalar_tensor_tensor)
    GPSIMD: tw = rr * w            (off the critical ring)

Sharding: data-parallel over batch, 1 batch element per NeuronCore (8 cores).
Layouts on chip are column-form: S=512 lives as [128 partitions x 4 free].
"""

import sys
import os
from contextlib import ExitStack

import numpy as np

for _p in ("/opt/trn_rl_repo",):
    if _p not in sys.path and os.path.isdir(_p):
        sys.path.insert(0, _p)

B, T, E, S = 8, 2048, 1024, 512
LN_EPS = 1e-5
NCORES = 8
UNROLL = 256

# block-upper-triangular tile order (ki = contraction chunk, m = output chunk)
TILE_ORDER = [(ki, m) for m in range(4) for ki in range(m + 1)]
N_PRE_TILES = 3   # tiles issued before the stats matmuls

_CACHE = {}


def build(t_len=T, unroll=UNROLL):
    """Build the single-core Bass program (SPMD across 8 cores)."""
    import concourse.bass as bass
    import concourse.bacc as bacc
    from concourse import mybir
    from concourse.tile import TileContext
    from concourse.tile_rust import add_dep_helper

    f32 = mybir.dt.float32
    bf16 = mybir.dt.bfloat16
    AF = mybir.ActivationFunctionType
    ALU = mybir.AluOpType
    ds = bass.ds

    n_iters = t_len // unroll
    assert n_iters * unroll == t_len and unroll % 2 == 0
    n_tc = t_len // 512 if t_len >= 512 else 1   # pre-pass t-chunks
    tcw = min(512, t_len)                        # pre-pass chunk width
    pcw = min(128, t_len)                        # post-pass chunk width
    n_pc = (t_len + pcw - 1) // pcw              # post-pass t-chunks

    nc = bacc.Bacc(trn_type="TRN2")

    xt = nc.dram_tensor("xt", [E, t_len], f32, kind="ExternalInput")
    wu = nc.dram_tensor("wu", [8, 4, 128, 128], f32, kind="ExternalInput")
    tt = nc.dram_tensor("tt", [len(TILE_ORDER), 128, 128], bf16, kind="ExternalInput")
    wo = nc.dram_tensor("wo", [S, E], bf16, kind="ExternalInput")
    bud = nc.dram_tensor("buc", [128, 4], f32, kind="ExternalInput")
    bod = nc.dram_tensor("bo", [1, E], f32, kind="ExternalInput")
    cnegd = nc.dram_tensor("cneg", [128, 4], f32, kind="ExternalInput")
    onesd = nc.dram_tensor("ones", [128, 128], bf16, kind="ExternalInput")
    y = nc.dram_tensor("y", [t_len, E], f32, kind="ExternalOutput")

    with ExitStack() as ctx:
        tc = ctx.enter_context(TileContext(nc))
        singles = ctx.enter_context(tc.tile_pool(name="singles", bufs=1))
        xpool = ctx.enter_context(tc.tile_pool(name="xpool", bufs=16))
        psum_big = ctx.enter_context(tc.tile_pool(name="psum_big", bufs=2, space="PSUM"))
        psum_w = ctx.enter_context(tc.tile_pool(name="psum_w", bufs=3, space="PSUM"))
        psum_s = ctx.enter_context(tc.tile_pool(name="psum_s", bufs=3, space="PSUM"))
        opool = ctx.enter_context(tc.tile_pool(name="opool", bufs=2))

        # ---- resident weights / constants ----
        wu_sb = singles.tile([128, 8, 4, 128], f32)
        nc.sync.dma_start(out=wu_sb, in_=wu.rearrange("k m p q -> p k m q"))
        tt_sbs = []
        for i in range(len(TILE_ORDER)):
            t_sb = singles.tile([128, 128], bf16, tag=f"tt{i}")
            nc.sync.dma_start(out=t_sb, in_=tt[i])
            tt_sbs.append(t_sb)
        wo_sb = singles.tile([128, 4, E], bf16)
        nc.sync.dma_start(out=wo_sb, in_=wo.rearrange("(k p) e -> p k e", p=128))
        bu_sb = singles.tile([128, 4], f32)
        nc.sync.dma_start(out=bu_sb, in_=bud[:])
        ones_sb = singles.tile([128, 128], bf16)
        nc.sync.dma_start(out=ones_sb, in_=onesd[:])
        bo_ap = bod[:]
        bo_sb = singles.tile([128, E], f32)
        nc.sync.dma_start(
            out=bo_sb,
            in_=bass.AP(tensor=bo_ap.tensor, offset=bo_ap.offset, ap=[[0, 128], [1, E]]),
        )
        cneg_sb = singles.tile([128, 4], f32)
        nc.sync.dma_start(out=cneg_sb, in_=cnegd[:])
        eps_sb = singles.tile([128, 1], f32)
        nc.vector.memset(eps_sb, LN_EPS)

        u_col = singles.tile([128, (t_len + 1) * 4], f32)
        states = singles.tile([128, t_len * 4], bf16)
        u_view = u_col.rearrange("p (t f) -> p t f", f=4)
        st_view = states.rearrange("p (t f) -> p t f", f=4)
        nc.vector.memset(u_col[:, t_len * 4:(t_len + 1) * 4], 0.0)

        # ---- pre-pass: uc_col[s, t] = (x @ W_u2 + b_u2).T in column form ----
        for c in range(n_tc):
            xts = []
            for e in range(8):
                xt_t = xpool.tile([128, tcw], f32, tag="xt")
                nc.sync.dma_start(
                    out=xt_t, in_=xt[e * 128:(e + 1) * 128, c * tcw:(c + 1) * tcw]
                )
                xts.append(xt_t)
            for m in range(4):
                ps = psum_big.tile([128, tcw], f32)
                for k in range(8):
                    nc.tensor.matmul(
                        ps, wu_sb[:, k, m, :], xts[k], start=(k == 0), stop=(k == 7)
                    )
                nc.scalar.activation(
                    out=u_view[:, c * tcw:(c + 1) * tcw, m],
                    in_=ps,
                    func=AF.Identity,
                    bias=bu_sb[:, m:m + 1],
                    scale=1.0,
                )

        # ---- scan ----
        w_a = singles.tile([128, 4], bf16)
        w_b = singles.tile([128, 4], bf16)
        wsq_a = singles.tile([128, 4], bf16)
        wsq_b = singles.tile([128, 4], bf16)
        rb_a = singles.tile([128, 1], f32)
        rb_b = singles.tile([128, 1], f32)
        # staging tiles: one dynamic DMA per loop iteration instead of one
        # register-offset AP per step (engines run out of registers past ~6)
        u_stg = singles.tile([128, (unroll + 1) * 4], f32)
        st_stg = singles.tile([128, unroll * 4], bf16)

        # prologue: w_0 = uc_0 + cneg (state at t=-1 is exactly zero, so the
        # beta-fold baked into b_u2 must be removed for step 0)
        nc.vector.tensor_add(w_a, u_col[:, 0:4], cneg_sb)

        def scan_step(jj):
            even = jj % 2 == 0
            cur, nxt = (w_a, w_b) if even else (w_b, w_a)
            rb = rb_a if even else rb_b
            wsq = wsq_a if even else wsq_b
            # squares for the variance (DVE, feeds the stats matmuls)
            nc.vector.tensor_mul(wsq, cur, cur)
            # early matvec tiles run while DVE computes wsq
            wp = psum_w.tile([128, 4], f32)
            pre_last = None
            for (ki, m) in TILE_ORDER[:N_PRE_TILES]:
                pre_last = nc.tensor.matmul(
                    wp[:, m:m + 1], tt_sbs[TILE_ORDER.index((ki, m))],
                    cur[:, ki:ki + 1], start=(ki == 0), stop=(ki == m),
                )
            # stats: Sum_p wsq broadcast to all partitions, accumulated over
            # the 4 column chunks into a single PSUM column
            sp = psum_s.tile([128, 1], f32)
            st_first = None
            st_last = None
            for kk in range(4):
                mm = nc.tensor.matmul(
                    sp, ones_sb, wsq[:, kk:kk + 1], start=(kk == 0), stop=(kk == 3),
                    skip_group_check=True,
                )
                if kk == 0:
                    st_first = mm
                st_last = mm
            add_dep_helper(st_first.ins, pre_last.ins, sync=False,
                           reason="stats after early tiles")
            # remaining matvec tiles run while ACT computes rr
            post_first = None
            for (ki, m) in TILE_ORDER[N_PRE_TILES:]:
                mm = nc.tensor.matmul(
                    wp[:, m:m + 1], tt_sbs[TILE_ORDER.index((ki, m))],
                    cur[:, ki:ki + 1], start=(ki == 0), stop=(ki == m),
                )
                if post_first is None:
                    post_first = mm
                    add_dep_helper(post_first.ins, st_last.ins, sync=False,
                                   reason="late tiles after stats")
            # rr = rsqrt(var + eps) straight from PSUM
            nc.scalar.activation(
                out=rb, in_=sp, func=AF.Abs_reciprocal_sqrt,
                bias=eps_sb, scale=1.0 / S,
            )
            # whitened state tw = rr*w (GPSIMD, off the critical ring)
            nc.gpsimd.tensor_scalar(
                out=st_stg[:, jj * 4:(jj + 1) * 4], in0=cur, scalar1=rb,
                scalar2=1.0, op0=ALU.mult, op1=ALU.mult,
            )
            # serial tail: w_{k+1} = rr*W + uc[k+1]
            nc.vector.scalar_tensor_tensor(
                out=nxt, in0=wp, scalar=rb, in1=u_stg[:, (jj + 1) * 4:(jj + 2) * 4],
                op0=ALU.mult, op1=ALU.add,
            )

        ucw = unroll * 4
        with tc.For_i(0, n_iters, 1, hint_engines=(
                mybir.EngineType.PE, mybir.EngineType.DVE,
                mybir.EngineType.Activation)) as iv:
            # stage uc[k] for k in [iv*unroll .. iv*unroll+unroll] (shifted +1
            # step: the STT of step jj reads uc[iv*unroll+jj+1])
            nc.gpsimd.dma_start(out=u_stg, in_=u_col[:, ds(iv * ucw, ucw + 4)])
            for jj in range(unroll):
                scan_step(jj)
            # flush whitened states for this chunk
            nc.gpsimd.dma_start(out=states[:, ds(iv * ucw, ucw)], in_=st_stg)

        # ---- post-pass: out = states @ W_O + b_out ----
        for t_i in range(n_pc):
            ob = opool.tile([128, E], f32)
            for ec in range(2):
                ps = psum_big.tile([128, 512], f32)
                for kk in range(4):
                    nc.tensor.matmul(
                        ps[:pcw, :],
                        st_view[:, t_i * pcw:(t_i + 1) * pcw, kk],
                        wo_sb[:, kk, ec * 512:(ec + 1) * 512],
                        start=(kk == 0),
                        stop=(kk == 3),
                    )
                nc.vector.tensor_add(
                    ob[:pcw, ec * 512:(ec + 1) * 512], ps[:pcw, :],
                    bo_sb[:pcw, ec * 512:(ec + 1) * 512]
                )
            nc.sync.dma_start(out=y[t_i * pcw:(t_i + 1) * pcw, :], in_=ob[:pcw, :])

    nc.compile()
    return nc


def _fix_boundaries(Tm, Q, bounds=(128, 256, 384)):
    """Thread 1x1 Schur blocks to the tile boundaries so no 2x2 block
    straddles a multiple of 128 (dtrexc keeps the similarity orthogonal)."""
    from scipy.linalg import lapack

    n = Tm.shape[0]

    def block_starts():
        starts, i = [], 0
        while i < n:
            if i + 1 < n and abs(Tm[i + 1, i]) > 1e-12:
                starts.append((i, 2)); i += 2
            else:
                starts.append((i, 1)); i += 1
        return starts

    for b in bounds:
        tries = 0
        banned = set()
        while abs(Tm[b, b - 1]) > 1e-12 and tries < 64:
            tries += 1
            ones = [p for p, sz in block_starts() if sz == 1 and p not in banned]
            if not ones:
                raise RuntimeError("no usable 1x1 Schur blocks")
            p = min(ones, key=lambda q: abs(q - b))
            if p > b:
                ifst, ilst = p + 1, b + 1
            else:
                ifst, ilst = p + 1, b
            Tm2, Q2, info = lapack.dtrexc(Tm, Q, ifst, ilst)
            if info != 0:
                banned.add(p)
                continue
            Tm, Q = Tm2, Q2
        if abs(Tm[b, b - 1]) > 1e-12:
            raise RuntimeError(f"could not clear Schur 2x2 straddle at {b}")
    return Tm, Q


def host_prep(inputs, t_len=T):
    """Fold parameters on the host; returns (shared dict, per-core xt list)."""
    from ml_dtypes import bfloat16
    import scipy.linalg as sla

    et = np.asarray(inputs["embedded_tokens"], np.float32)
    W_e2s = np.asarray(inputs["W_e2s"], np.float64)
    b_e2s = np.asarray(inputs["b_e2s"], np.float64)
    A = np.asarray(inputs["A"], np.float64)
    Bm = np.asarray(inputs["Bm"], np.float64)
    C = np.asarray(inputs["C"], np.float64)
    gamma = np.asarray(inputs["ln_gamma"], np.float64)
    beta = np.asarray(inputs["ln_beta"], np.float64)
    W_s2o = np.asarray(inputs["W_s2o"], np.float64)
    b_s2o = np.asarray(inputs["b_s2o"], np.float64)

    G = gamma[:, None] * A
    Gt = G - np.outer(G @ np.ones(S) / S, np.ones(S))   # zero row-sums
    Tm, Q = sla.schur(Gt, output="real")
    Tm, Q = _fix_boundaries(Tm, Q)
    for ki in range(4):
        for kj in range(4):
            if ki > kj:
                Tm[128 * ki:128 * ki + 128, 128 * kj:128 * kj + 128] = 0.0
    tt_tiles = np.stack([
        Tm[128 * ki:128 * ki + 128, 128 * m:128 * m + 128]
        for (ki, m) in TILE_ORDER
    ])

    CS = np.eye(S) - np.ones((S, S)) / S                 # centering matrix
    W_u2 = (W_e2s @ Bm) @ CS @ Q                         # [E, S]
    b_u2 = ((b_e2s @ Bm + beta @ A) @ CS) @ Q            # [S]
    cneg = -(((beta @ A) @ CS) @ Q)                      # step-0 fix
    W_O = Q.T @ (gamma[:, None] * C) @ W_s2o             # [S, E]
    b_out = beta @ C @ W_s2o + b_s2o                     # [E]

    wu_tiles = np.ascontiguousarray(
        W_u2.astype(np.float32).reshape(8, 128, 4, 128).transpose(0, 2, 1, 3)
    )  # [k, m, 128, 128]

    shared = {
        "wu": wu_tiles,
        "tt": np.ascontiguousarray(tt_tiles.astype(bfloat16)),
        "wo": np.ascontiguousarray(W_O.astype(bfloat16)),
        "buc": np.ascontiguousarray(b_u2.astype(np.float32).reshape(4, 128).T),
        "bo": np.ascontiguousarray(b_out.astype(np.float32).reshape(1, E)),
        "cneg": np.ascontiguousarray(cneg.astype(np.float32).reshape(4, 128).T),
        "ones": np.ones((128, 128), bfloat16),
    }
    xts = [
        np.ascontiguousarray(et[b, :t_len, :].T.astype(np.float32))
        for b in range(et.shape[0])
    ]
    return shared, xts


def kernel(**inputs):
    key = ("nc", T, UNROLL)
    if key not in _CACHE:
        _CACHE[key] = build(T, UNROLL)
    nc = _CACHE[key]

    from concourse.bass_utils import run_bass_kernel_spmd

    shared, xts = host_prep(inputs)
    in_maps = [dict(shared, xt=xts[b]) for b in range(B)]
    res = run_bass_kernel_spmd(nc, in_maps, core_ids=list(range(NCORES)))
    out = np.stack([np.asarray(r["y"], np.float32) for r in res.results], axis=0)
    return out


# revision 11
# speedup vs baseline: 1.6170x; 1.0532x over previous
"""Trainium2 Bass kernel for the LayerNorm-RNN attention variant.

Math (per batch element b, reference semantics):
    u_t   = (x_t @ W_e2s + b_e2s) @ Bm
    y_t   = s_{t-1} @ A + u_t
    s_t   = LN(y_t) * gamma + beta
    out_t = (s_t @ C) @ W_s2o + b_s2o

Key reformulation (all folds exact, done host-side in fp64):
  1. Mean-free weights: G = diag(gamma) @ A,  Gt = G - (G@1/S) 1^T has zero
     row-sums, so W = zc @ Gt is exactly zero-mean for any zc. Tracking the
     CENTERED pre-norm state zc kills the per-step mean/bias bookkeeping:
         zc_{t+1} = rr_t * (zc_t @ Gt) + uc_{t+1},   rr_t = rsqrt(|zc_t|^2/S + eps)
     with uc = centered input injection (centering matrix folded into W_u).
  2. Orthogonal Schur basis: Gt = Q T Q^T (real Schur, 2x2 blocks nudged off
     the 128-boundaries with dtrexc). w = zc @ Q keeps |w| = |zc| (stats
     unchanged) while T is block-upper-triangular: the per-step matvec needs
     only 10 of 16 [128,128] tiles.
  3. Whitened states tw_t = rr_t * w_t are accumulated and folded through
     W_O = Q^T diag(gamma) C W_s2o in a bulk post-pass.

Per-step engine schedule (the serial ring is the wall clock; everything else
hides inside it):
    DVE:  wsq = w*w
    PE :  3 early T tiles | stats: Sum_p wsq/S via 4 accumulating matmuls with
          a 1/S stationary (broadcast to all partitions) | 7 late tiles
    ACT:  rr = rsqrt(var + eps)  directly from PSUM
    DVE:  w' = rr * W + uc_next    (single scalar_tensor_tensor)
    GPSIMD: tw = rr * w            (off the critical ring)

The scan is fully unrolled in Python (no hardware loop); the input pre-pass
(x @ W_u2 chunks) and output post-pass (states @ W_O chunks) are sprinkled
into the PE/ACT idle windows of the scan so they cost ~no wall clock.

Sharding: data-parallel over batch, 1 batch element per NeuronCore (8 cores).
Layouts on chip are column-form: S=512 lives as [128 partitions x 4 free].
"""

import sys
import os
from contextlib import ExitStack

import numpy as np

for _p in ("/opt/trn_rl_repo",):
    if _p not in sys.path and os.path.isdir(_p):
        sys.path.insert(0, _p)

B, T, E, S = 8, 2048, 1024, 512
LN_EPS = 1e-5
NCORES = 8

# block-upper-triangular tile order (ki = contraction chunk, m = output chunk)
TILE_ORDER = [(ki, m) for m in range(4) for ki in range(m + 1)]
N_PRE_TILES = 3     # tiles issued before the stats matmuls
PRE_CHUNK = 512     # pre-pass t-chunk width
POST_CHUNK = 128    # post-pass t-chunk width
FILLER_EVERY = 4    # emit one filler work item every this many scan steps

_CACHE = {}


def build(t_len=T):
    """Build the single-core Bass program (SPMD across 8 cores)."""
    import concourse.bass as bass
    import concourse.bacc as bacc
    from concourse import mybir
    from concourse.tile import TileContext
    from concourse.tile_rust import add_dep_helper

    f32 = mybir.dt.float32
    bf16 = mybir.dt.bfloat16
    AF = mybir.ActivationFunctionType
    ALU = mybir.AluOpType

    n_tc = (t_len + PRE_CHUNK - 1) // PRE_CHUNK
    tcw = min(PRE_CHUNK, t_len)
    pcw = min(POST_CHUNK, t_len)
    n_pc = (t_len + pcw - 1) // pcw

    nc = bacc.Bacc(trn_type="TRN2")

    xt = nc.dram_tensor("xt", [E, t_len], f32, kind="ExternalInput")
    wu = nc.dram_tensor("wu", [8, 4, 128, 128], f32, kind="ExternalInput")
    tt = nc.dram_tensor("tt", [len(TILE_ORDER), 128, 128], bf16, kind="ExternalInput")
    wo = nc.dram_tensor("wo", [S, E], bf16, kind="ExternalInput")
    bud = nc.dram_tensor("buc", [128, 4], f32, kind="ExternalInput")
    bo4d = nc.dram_tensor("bo4", [1, E], bf16, kind="ExternalInput")
    cnegd = nc.dram_tensor("cneg", [128, 4], f32, kind="ExternalInput")
    onesd = nc.dram_tensor("ones", [128, 128], bf16, kind="ExternalInput")
    y = nc.dram_tensor("y", [t_len, E], f32, kind="ExternalOutput")

    with ExitStack() as ctx:
        tc = ctx.enter_context(TileContext(nc))
        singles = ctx.enter_context(tc.tile_pool(name="singles", bufs=1))
        xpool = ctx.enter_context(tc.tile_pool(name="xpool", bufs=16))
        psum_big = ctx.enter_context(tc.tile_pool(name="psum_big", bufs=2, space="PSUM"))
        psum_w = ctx.enter_context(tc.tile_pool(name="psum_w", bufs=2, space="PSUM"))
        psum_s = ctx.enter_context(tc.tile_pool(name="psum_s", bufs=2, space="PSUM"))
        opool = ctx.enter_context(tc.tile_pool(name="opool", bufs=2))

        # ---- resident weights / constants ----
        wu_sb = singles.tile([128, 8, 4, 128], f32)
        nc.sync.dma_start(out=wu_sb, in_=wu.rearrange("k m p q -> p k m q"))
        tt_sbs = []
        for i in range(len(TILE_ORDER)):
            t_sb = singles.tile([128, 128], bf16, tag=f"tt{i}")
            nc.sync.dma_start(out=t_sb, in_=tt[i])
            tt_sbs.append(t_sb)
        wo_sb = singles.tile([128, 4, E], bf16)
        nc.sync.dma_start(out=wo_sb, in_=wo.rearrange("(k p) e -> p k e", p=128))
        bu_sb = singles.tile([128, 4], f32)
        nc.sync.dma_start(out=bu_sb, in_=bud[:])
        ones_sb = singles.tile([128, 128], bf16)
        nc.sync.dma_start(out=ones_sb, in_=onesd[:])
        bo4_ap = bo4d[:]
        bo4_sb = singles.tile([128, E], bf16)
        nc.sync.dma_start(
            out=bo4_sb,
            in_=bass.AP(tensor=bo4_ap.tensor, offset=bo4_ap.offset, ap=[[0, 128], [1, E]]),
        )
        cneg_sb = singles.tile([128, 4], f32)
        nc.sync.dma_start(out=cneg_sb, in_=cnegd[:])
        eps_sb = singles.tile([128, 1], f32)
        nc.vector.memset(eps_sb, LN_EPS)

        u_col = singles.tile([128, (t_len + 1) * 4], f32)
        states = singles.tile([128, t_len * 4], bf16)
        u_view = u_col.rearrange("p (t f) -> p t f", f=4)
        st_view = states.rearrange("p (t f) -> p t f", f=4)
        nc.vector.memset(u_col[:, t_len * 4:(t_len + 1) * 4], 0.0)

        # ---- pre-pass emitter: uc[t-chunk] = (x @ W_u2).T + b_u2, col form ----
        evac_insts = {}   # chunk -> list of evacuation ACT instructions

        def pre_pass_items(c):
            """Return filler callables computing u_col for t-chunk c."""
            xts = [None] * 8
            items = []
            evac_insts[c] = []

            def load_x():
                for e in range(8):
                    xts[e] = xpool.tile([128, tcw], f32, tag="xt", name="xtile")
                    nc.sync.dma_start(
                        out=xts[e],
                        in_=xt[e * 128:(e + 1) * 128, c * tcw:(c + 1) * tcw],
                    )
                return None
            items.append(load_x)
            for m in range(4):
                for h in range(2):
                    ps_box = [None]

                    def mk_mm(m=m, k=0, h=h, ps_box=ps_box):
                        def mm():
                            if ps_box[0] is None:
                                ps_box[0] = psum_big.tile([128, 256], f32, tag="pre", name="pre_ps")
                            return nc.tensor.matmul(
                                ps_box[0],
                                wu_sb[:, k, m, :],
                                xts[k][:, h * 256:(h + 1) * 256],
                                start=(k == 0), stop=(k == 7),
                            )
                        return mm

                    def mk_evac(m=m, h=h, ps_box=ps_box):
                        def evac():
                            inst = nc.scalar.activation(
                                out=u_view[:, c * tcw + h * 256:c * tcw + (h + 1) * 256, m],
                                in_=ps_box[0],
                                func=AF.Identity, bias=bu_sb[:, m:m + 1], scale=1.0,
                            )
                            evac_insts[c].append(inst)
                            return inst
                        return evac
                    for k in range(8):
                        items.append(mk_mm(m=m, k=k, h=h, ps_box=ps_box))
                    items.append(mk_evac(m=m, h=h, ps_box=ps_box))
            return items

        # ---- post-pass emitter: y[t-chunk] = states @ W_O + b_out ----
        def post_pass_items(t_i):
            ob_box = [None]
            items = []

            def mk_mm(ec=0, h=0, kk=0, ps_box=None):
                def mm():
                    if ob_box[0] is None:
                        ob_box[0] = opool.tile([128, E], f32, name="ob")
                    if ps_box[0] is None:
                        ps_box[0] = psum_big.tile([128, 256], f32, tag="post", name="post_ps")
                    lo = ec * 512 + h * 256
                    if kk < 0:   # bias seed: (1/S ones)^T @ (4*b_out) = b_out
                        inst = nc.tensor.matmul(
                            ps_box[0], ones_sb, bo4_sb[:, lo:lo + 256],
                            start=True, stop=False,
                        )
                        guard = gp_insts[min((t_i + 1) * pcw, t_len) - 1]
                        if guard is not None:
                            add_dep_helper(inst.ins, guard.ins, sync=True,
                                           reason="post-pass waits for states chunk")
                        return inst
                    return nc.tensor.matmul(
                        ps_box[0],
                        st_view[:, t_i * pcw:(t_i + 1) * pcw, kk],
                        wo_sb[:, kk, lo:lo + 256],
                        start=False, stop=(kk == 3),
                    )
                return mm

            def mk_copy(ec=0, h=0, ps_box=None):
                def cp():
                    lo = ec * 512 + h * 256
                    return nc.scalar.activation(
                        out=ob_box[0][:pcw, lo:lo + 256], in_=ps_box[0][:pcw, :],
                        func=AF.Identity, scale=1.0,
                    )
                return cp

            for ec in range(2):
                for h in range(2):
                    ps_box = [None]
                    for kk in (-1, 0, 1, 2, 3):
                        items.append(mk_mm(ec=ec, h=h, kk=kk, ps_box=ps_box))
                    items.append(mk_copy(ec=ec, h=h, ps_box=ps_box))

            def store():
                nc.sync.dma_start(
                    out=y[t_i * pcw:(t_i + 1) * pcw, :], in_=ob_box[0][:pcw, :]
                )
                return None
            items.append(store)
            return items

        # ---- scan state ----
        w_a = singles.tile([128, 4], bf16)
        w_b = singles.tile([128, 4], bf16)
        wsq_a = singles.tile([128, 4], bf16)
        wsq_b = singles.tile([128, 4], bf16)
        rb_a = singles.tile([128, 1], f32)
        rb_b = singles.tile([128, 1], f32)

        # chunk 0 of the pre-pass runs up front (the scan needs it immediately)
        for item in pre_pass_items(0):
            item()

        # prologue: w_0 = uc_0 + cneg (state at t=-1 is exactly zero, so the
        # beta-fold baked into b_u2 must be removed for step 0)
        nc.vector.tensor_add(w_a, u_col[:, 0:4], cneg_sb)

        # filler queue: (step at which the work becomes legal, items)
        # pre-pass chunks depend only on DMAs, so schedule them as early as
        # xpool capacity allows -- they must finish WELL before the scan
        # reaches them (the chunk-boundary STT also takes explicit deps)
        filler = []
        for c in range(1, n_tc):
            filler.append(((c - 1) * 290 + 2, pre_pass_items(c)))
        for t_i in range(n_pc - 1):
            filler.append(((t_i + 1) * pcw + 2, post_pass_items(t_i)))
        filler.sort(key=lambda x: x[0])

        last_tile_box = [None]
        gp_insts = [None] * t_len

        def scan_step(jj):
            even = jj % 2 == 0
            cur, nxt = (w_a, w_b) if even else (w_b, w_a)
            rb = rb_a if even else rb_b
            wsq = wsq_a if even else wsq_b
            # squares for the variance (DVE, feeds the stats matmuls)
            nc.vector.tensor_mul(wsq, cur, cur)
            # early matvec tiles run while DVE computes wsq
            wp = psum_w.tile([128, 4], f32)
            pre_last = None
            for (ki, m) in TILE_ORDER[:N_PRE_TILES]:
                pre_last = nc.tensor.matmul(
                    wp[:, m:m + 1], tt_sbs[TILE_ORDER.index((ki, m))],
                    cur[:, ki:ki + 1], start=(ki == 0), stop=(ki == m),
                )
            # stats: Sum_p wsq/S broadcast to all partitions, accumulated over
            # the 4 column chunks into a single PSUM column
            sp = psum_s.tile([128, 1], f32)
            st_first = None
            st_last = None
            for kk in range(4):
                mm = nc.tensor.matmul(
                    sp, ones_sb, wsq[:, kk:kk + 1], start=(kk == 0), stop=(kk == 3),
                    skip_group_check=True,
                )
                if kk == 0:
                    st_first = mm
                st_last = mm
            add_dep_helper(st_first.ins, pre_last.ins, sync=False,
                           reason="stats after early tiles")
            # remaining matvec tiles run while ACT computes rr
            post_first = None
            for (ki, m) in TILE_ORDER[N_PRE_TILES:]:
                mm = nc.tensor.matmul(
                    wp[:, m:m + 1], tt_sbs[TILE_ORDER.index((ki, m))],
                    cur[:, ki:ki + 1], start=(ki == 0), stop=(ki == m),
                )
                if post_first is None:
                    post_first = mm
                    add_dep_helper(post_first.ins, st_last.ins, sync=False,
                                   reason="late tiles after stats")
                last_tile_box[0] = mm
            # rr = rsqrt(var + eps) straight from PSUM (1/S is in the ones)
            nc.scalar.activation(
                out=rb, in_=sp, func=AF.Abs_reciprocal_sqrt,
                bias=eps_sb, scale=1.0,
            )
            # whitened state tw = rr*w (GPSIMD, off the critical ring)
            gp = nc.gpsimd.tensor_scalar(
                out=st_view[:, jj, :], in0=cur, scalar1=rb,
                scalar2=1.0, op0=ALU.mult, op1=ALU.mult,
            )
            gp_insts[jj] = gp
            # serial tail: w_{k+1} = rr*W + uc[k+1]
            stt = nc.vector.scalar_tensor_tensor(
                out=nxt, in0=wp, scalar=rb, in1=u_view[:, jj + 1, :],
                op0=ALU.mult, op1=ALU.add,
            )
            # the STT that first consumes a pre-pass chunk must wait for all
            # of that chunk's evacuations (the strided-slice RAW dep is not
            # reliably auto-tracked)
            if (jj + 1) % PRE_CHUNK == 0 and (jj + 1) // PRE_CHUNK in evac_insts:
                evs = evac_insts[(jj + 1) // PRE_CHUNK]
                assert len(evs) == 8, (
                    f"pre-pass chunk {(jj + 1) // PRE_CHUNK} only has "
                    f"{len(evs)}/8 evacuations emitted by step {jj}"
                )
                for ev in evs:
                    add_dep_helper(stt.ins, ev.ins, sync=True,
                                   reason="scan waits for pre-pass chunk")

        fill_idx = 0
        cur_items = []
        for jj in range(t_len):
            scan_step(jj)
            if not cur_items and fill_idx < len(filler) and jj >= filler[fill_idx][0]:
                cur_items = list(filler[fill_idx][1])
                fill_idx += 1
            if cur_items and jj % FILLER_EVERY == 0:
                inst = cur_items.pop(0)()
                if inst is not None and last_tile_box[0] is not None:
                    iobj = inst.ins if hasattr(inst, "ins") else inst
                    add_dep_helper(iobj, last_tile_box[0].ins, sync=False,
                                   reason="filler after scan tiles")

        # leftover filler (tail post-pass chunks) runs after the scan
        while cur_items or fill_idx < len(filler):
            if not cur_items and fill_idx < len(filler):
                cur_items = list(filler[fill_idx][1])
                fill_idx += 1
            if cur_items:
                cur_items.pop(0)()
        for item in post_pass_items(n_pc - 1):
            item()

    nc.compile()
    return nc


def _fix_boundaries(Tm, Q, bounds=(128, 256, 384)):
    """Thread 1x1 Schur blocks to the tile boundaries so no 2x2 block
    straddles a multiple of 128 (dtrexc keeps the similarity orthogonal)."""
    from scipy.linalg import lapack

    n = Tm.shape[0]

    def block_starts():
        starts, i = [], 0
        while i < n:
            if i + 1 < n and abs(Tm[i + 1, i]) > 1e-12:
                starts.append((i, 2)); i += 2
            else:
                starts.append((i, 1)); i += 1
        return starts

    for b in bounds:
        tries = 0
        banned = set()
        while abs(Tm[b, b - 1]) > 1e-12 and tries < 64:
            tries += 1
            ones = [p for p, sz in block_starts() if sz == 1 and p not in banned]
            if not ones:
                raise RuntimeError("no usable 1x1 Schur blocks")
            p = min(ones, key=lambda q: abs(q - b))
            if p > b:
                ifst, ilst = p + 1, b + 1
            else:
                ifst, ilst = p + 1, b
            Tm2, Q2, info = lapack.dtrexc(Tm, Q, ifst, ilst)
            if info != 0:
                banned.add(p)
                continue
            Tm, Q = Tm2, Q2
        if abs(Tm[b, b - 1]) > 1e-12:
            raise RuntimeError(f"could not clear Schur 2x2 straddle at {b}")
    return Tm, Q


def host_prep(inputs, t_len=T):
    """Fold parameters on the host; returns (shared dict, per-core xt list)."""
    from ml_dtypes import bfloat16
    import scipy.linalg as sla

    et = np.asarray(inputs["embedded_tokens"], np.float32)
    W_e2s = np.asarray(inputs["W_e2s"], np.float64)
    b_e2s = np.asarray(inputs["b_e2s"], np.float64)
    A = np.asarray(inputs["A"], np.float64)
    Bm = np.asarray(inputs["Bm"], np.float64)
    C = np.asarray(inputs["C"], np.float64)
    gamma = np.asarray(inputs["ln_gamma"], np.float64)
    beta = np.asarray(inputs["ln_beta"], np.float64)
    W_s2o = np.asarray(inputs["W_s2o"], np.float64)
    b_s2o = np.asarray(inputs["b_s2o"], np.float64)

    G = gamma[:, None] * A
    Gt = G - np.outer(G @ np.ones(S) / S, np.ones(S))   # zero row-sums
    Tm, Q = sla.schur(Gt, output="real")
    Tm, Q = _fix_boundaries(Tm, Q)
    for ki in range(4):
        for kj in range(4):
            if ki > kj:
                Tm[128 * ki:128 * ki + 128, 128 * kj:128 * kj + 128] = 0.0
    tt_tiles = np.stack([
        Tm[128 * ki:128 * ki + 128, 128 * m:128 * m + 128]
        for (ki, m) in TILE_ORDER
    ])

    CS = np.eye(S) - np.ones((S, S)) / S                 # centering matrix
    W_u2 = (W_e2s @ Bm) @ CS @ Q                         # [E, S]
    b_u2 = ((b_e2s @ Bm + beta @ A) @ CS) @ Q            # [S]
    cneg = -(((beta @ A) @ CS) @ Q)                      # step-0 fix
    W_O = Q.T @ (gamma[:, None] * C) @ W_s2o             # [S, E]
    b_out = beta @ C @ W_s2o + b_s2o                     # [E]

    wu_tiles = np.ascontiguousarray(
        W_u2.astype(np.float32).reshape(8, 128, 4, 128).transpose(0, 2, 1, 3)
    )  # [k, m, 128, 128]

    shared = {
        "wu": wu_tiles,
        "tt": np.ascontiguousarray(tt_tiles.astype(bfloat16)),
        "wo": np.ascontiguousarray(W_O.astype(bfloat16)),
        "buc": np.ascontiguousarray(b_u2.astype(np.float32).reshape(4, 128).T),
        # bias seeded through the 1/S-ones matmul: sum_p (1/S)*(4*b_out) = b_out
        "bo4": np.ascontiguousarray((4.0 * b_out).astype(bfloat16).reshape(1, E)),
        "cneg": np.ascontiguousarray(cneg.astype(np.float32).reshape(4, 128).T),
        "ones": np.full((128, 128), 1.0 / S, bfloat16),
    }
    xts = [
        np.ascontiguousarray(et[b, :t_len, :].T.astype(np.float32))
        for b in range(et.shape[0])
    ]
    return shared, xts


def kernel(**inputs):
    key = ("nc", T)
    if key not in _CACHE:
        _CACHE[key] = build(T)
    nc = _CACHE[key]

    from concourse.bass_utils import run_bass_kernel_spmd

    shared, xts = host_prep(inputs)
    in_maps = [dict(shared, xt=xts[b]) for b in range(B)]
    res = run_bass_kernel_spmd(nc, in_maps, core_ids=list(range(NCORES)))
    out = np.stack([np.asarray(r["y"], np.float32) for r in res.results], axis=0)
    return out


# revision 12
# speedup vs baseline: 1.6178x; 1.0005x over previous
"""Trainium2 Bass kernel for the LayerNorm-RNN attention variant.

Math (per batch element b, reference semantics):
    u_t   = (x_t @ W_e2s + b_e2s) @ Bm
    y_t   = s_{t-1} @ A + u_t
    s_t   = LN(y_t) * gamma + beta
    out_t = (s_t @ C) @ W_s2o + b_s2o

Key reformulation (all folds exact, done host-side in fp64):
  1. Mean-free weights: G = diag(gamma) @ A,  Gt = G - (G@1/S) 1^T has zero
     row-sums, so W = zc @ Gt is exactly zero-mean for any zc. Tracking the
     CENTERED pre-norm state zc kills the per-step mean/bias bookkeeping:
         zc_{t+1} = rr_t * (zc_t @ Gt) + uc_{t+1},   rr_t = rsqrt(|zc_t|^2/S + eps)
     with uc = centered input injection (centering matrix folded into W_u).
  2. Orthogonal Schur basis: Gt = Q T Q^T (real Schur, 2x2 blocks nudged off
     the 128-boundaries with dtrexc). w = zc @ Q keeps |w| = |zc| (stats
     unchanged) while T is block-upper-triangular: the per-step matvec needs
     only 10 of 16 [128,128] tiles.
  3. Whitened states tw_t = rr_t * w_t are accumulated and folded through
     W_O = Q^T diag(gamma) C W_s2o in a bulk post-pass.

Per-step engine schedule (the serial ring is the wall clock; everything else
hides inside it):
    DVE:  wsq = w*w
    PE :  3 early T tiles | stats: Sum_p wsq/S via 4 accumulating matmuls with
          a 1/S stationary (broadcast to all partitions) | 7 late tiles
    ACT:  rr = rsqrt(var + eps)  directly from PSUM
    DVE:  w' = rr * W + uc_next    (single scalar_tensor_tensor)
    GPSIMD: tw = rr * w            (off the critical ring)

The scan is fully unrolled in Python (no hardware loop); the input pre-pass
(x @ W_u2 chunks) and output post-pass (states @ W_O chunks) are sprinkled
into the PE/ACT idle windows of the scan so they cost ~no wall clock.

Sharding: data-parallel over batch, 1 batch element per NeuronCore (8 cores).
Layouts on chip are column-form: S=512 lives as [128 partitions x 4 free].
"""

import sys
import os
from contextlib import ExitStack

import numpy as np

for _p in ("/opt/trn_rl_repo",):
    if _p not in sys.path and os.path.isdir(_p):
        sys.path.insert(0, _p)

B, T, E, S = 8, 2048, 1024, 512
LN_EPS = 1e-5
NCORES = 8

# block-upper-triangular tile order (ki = contraction chunk, m = output chunk)
TILE_ORDER = [(ki, m) for m in range(4) for ki in range(m + 1)]
N_PRE_TILES = 3     # tiles issued before the stats matmuls
PRE_CHUNK = 512     # pre-pass t-chunk width
POST_CHUNK = 128    # post-pass t-chunk width
FILLER_EVERY = 4    # emit one filler work item every this many scan steps

_CACHE = {}


def build(t_len=T):
    """Build the single-core Bass program (SPMD across 8 cores)."""
    import concourse.bass as bass
    import concourse.bacc as bacc
    from concourse import mybir
    from concourse.tile import TileContext
    from concourse.tile_rust import add_dep_helper

    f32 = mybir.dt.float32
    bf16 = mybir.dt.bfloat16
    AF = mybir.ActivationFunctionType
    ALU = mybir.AluOpType

    n_tc = (t_len + PRE_CHUNK - 1) // PRE_CHUNK
    tcw = min(PRE_CHUNK, t_len)
    pcw = min(POST_CHUNK, t_len)
    n_pc = (t_len + pcw - 1) // pcw

    nc = bacc.Bacc(trn_type="TRN2")

    xt = nc.dram_tensor("xt", [E, t_len], f32, kind="ExternalInput")
    wu = nc.dram_tensor("wu", [8, 4, 128, 128], f32, kind="ExternalInput")
    tt = nc.dram_tensor("tt", [len(TILE_ORDER), 128, 128], bf16, kind="ExternalInput")
    wo = nc.dram_tensor("wo", [S, E], bf16, kind="ExternalInput")
    bud = nc.dram_tensor("buc", [128, 4], f32, kind="ExternalInput")
    bo4d = nc.dram_tensor("bo4", [1, E], bf16, kind="ExternalInput")
    cnegd = nc.dram_tensor("cneg", [128, 4], f32, kind="ExternalInput")
    onesd = nc.dram_tensor("ones", [128, 128], bf16, kind="ExternalInput")
    y = nc.dram_tensor("y", [t_len, E], f32, kind="ExternalOutput")

    with ExitStack() as ctx:
        tc = ctx.enter_context(TileContext(nc))
        singles = ctx.enter_context(tc.tile_pool(name="singles", bufs=1))
        xpool = ctx.enter_context(tc.tile_pool(name="xpool", bufs=16))
        psum_big = ctx.enter_context(tc.tile_pool(name="psum_big", bufs=2, space="PSUM"))
        psum_w = ctx.enter_context(tc.tile_pool(name="psum_w", bufs=2, space="PSUM"))
        psum_s = ctx.enter_context(tc.tile_pool(name="psum_s", bufs=1, space="PSUM"))
        psum_d = ctx.enter_context(tc.tile_pool(name="psum_d", bufs=1, space="PSUM"))
        opool = ctx.enter_context(tc.tile_pool(name="opool", bufs=2))

        # ---- resident weights / constants ----
        wu_sb = singles.tile([128, 8, 4, 128], f32)
        nc.sync.dma_start(out=wu_sb, in_=wu.rearrange("k m p q -> p k m q"))
        tt_sbs = []
        for i in range(len(TILE_ORDER)):
            t_sb = singles.tile([128, 128], bf16, tag=f"tt{i}")
            nc.sync.dma_start(out=t_sb, in_=tt[i])
            tt_sbs.append(t_sb)
        wo_sb = singles.tile([128, 4, E], bf16)
        nc.sync.dma_start(out=wo_sb, in_=wo.rearrange("(k p) e -> p k e", p=128))
        bu_sb = singles.tile([128, 4], f32)
        nc.sync.dma_start(out=bu_sb, in_=bud[:])
        ones_sb = singles.tile([128, 128], bf16)
        nc.sync.dma_start(out=ones_sb, in_=onesd[:])
        bo4_ap = bo4d[:]
        bo4_sb = singles.tile([128, E], bf16)
        nc.sync.dma_start(
            out=bo4_sb,
            in_=bass.AP(tensor=bo4_ap.tensor, offset=bo4_ap.offset, ap=[[0, 128], [1, E]]),
        )
        cneg_sb = singles.tile([128, 4], f32)
        nc.sync.dma_start(out=cneg_sb, in_=cnegd[:])
        eps_sb = singles.tile([128, 1], f32)
        nc.vector.memset(eps_sb, LN_EPS)

        u_col = singles.tile([128, (t_len + 1) * 4], f32)
        states = singles.tile([128, t_len * 4], bf16)
        u_view = u_col.rearrange("p (t f) -> p t f", f=4)
        st_view = states.rearrange("p (t f) -> p t f", f=4)
        nc.vector.memset(u_col[:, t_len * 4:(t_len + 1) * 4], 0.0)

        # ---- pre-pass emitter: uc[t-chunk] = (x @ W_u2).T + b_u2, col form ----
        evac_insts = {}   # chunk -> list of evacuation ACT instructions

        def pre_pass_items(c):
            """Return filler callables computing u_col for t-chunk c."""
            xts = [None] * 8
            items = []
            evac_insts[c] = []

            def load_x():
                for e in range(8):
                    xts[e] = xpool.tile([128, tcw], f32, tag="xt", name="xtile")
                    nc.sync.dma_start(
                        out=xts[e],
                        in_=xt[e * 128:(e + 1) * 128, c * tcw:(c + 1) * tcw],
                    )
                return None
            items.append(load_x)
            for m in range(4):
                for h in range(2):
                    ps_box = [None]

                    def mk_mm(m=m, k=0, h=h, ps_box=ps_box):
                        def mm():
                            if ps_box[0] is None:
                                ps_box[0] = psum_big.tile([128, 256], f32, tag="pre", name="pre_ps")
                            return nc.tensor.matmul(
                                ps_box[0],
                                wu_sb[:, k, m, :],
                                xts[k][:, h * 256:(h + 1) * 256],
                                start=(k == 0), stop=(k == 7),
                            )
                        return mm

                    def mk_evac(m=m, h=h, ps_box=ps_box):
                        def evac():
                            inst = nc.scalar.activation(
                                out=u_view[:, c * tcw + h * 256:c * tcw + (h + 1) * 256, m],
                                in_=ps_box[0],
                                func=AF.Identity, bias=bu_sb[:, m:m + 1], scale=1.0,
                            )
                            evac_insts[c].append(inst)
                            return inst
                        return evac
                    for k in range(8):
                        items.append(mk_mm(m=m, k=k, h=h, ps_box=ps_box))
                    items.append(mk_evac(m=m, h=h, ps_box=ps_box))
            return items

        # ---- post-pass emitter: y[t-chunk] = states @ W_O + b_out ----
        def post_pass_items(t_i):
            ob_box = [None]
            items = []

            def mk_mm(ec=0, h=0, kk=0, ps_box=None):
                def mm():
                    if ob_box[0] is None:
                        ob_box[0] = opool.tile([128, E], f32, name="ob")
                    if ps_box[0] is None:
                        ps_box[0] = psum_big.tile([128, 256], f32, tag="post", name="post_ps")
                    lo = ec * 512 + h * 256
                    if kk < 0:   # bias seed: (1/S ones)^T @ (4*b_out) = b_out
                        inst = nc.tensor.matmul(
                            ps_box[0], ones_sb, bo4_sb[:, lo:lo + 256],
                            start=True, stop=False,
                        )
                        guard = gp_insts[min((t_i + 1) * pcw, t_len) - 1]
                        if guard is not None:
                            add_dep_helper(inst.ins, guard.ins, sync=True,
                                           reason="post-pass waits for states chunk")
                        return inst
                    return nc.tensor.matmul(
                        ps_box[0],
                        st_view[:, t_i * pcw:(t_i + 1) * pcw, kk],
                        wo_sb[:, kk, lo:lo + 256],
                        start=False, stop=(kk == 3),
                    )
                return mm

            def mk_copy(ec=0, h=0, ps_box=None):
                def cp():
                    lo = ec * 512 + h * 256
                    return nc.scalar.activation(
                        out=ob_box[0][:pcw, lo:lo + 256], in_=ps_box[0][:pcw, :],
                        func=AF.Identity, scale=1.0,
                    )
                return cp

            for ec in range(2):
                for h in range(2):
                    ps_box = [None]
                    for kk in (-1, 0, 1, 2, 3):
                        items.append(mk_mm(ec=ec, h=h, kk=kk, ps_box=ps_box))
                    items.append(mk_copy(ec=ec, h=h, ps_box=ps_box))

            def store():
                nc.sync.dma_start(
                    out=y[t_i * pcw:(t_i + 1) * pcw, :], in_=ob_box[0][:pcw, :]
                )
                return None
            items.append(store)
            return items

        # ---- scan state ----
        w_a = singles.tile([128, 4], bf16)
        w_b = singles.tile([128, 4], bf16)
        wsq_a = singles.tile([128, 4], bf16)
        wsq_b = singles.tile([128, 4], bf16)
        rb_a = singles.tile([128, 1], f32)
        rb_b = singles.tile([128, 1], f32)

        # chunk 0 of the pre-pass runs up front (the scan needs it immediately)
        for item in pre_pass_items(0):
            item()

        # prologue: w_0 = uc_0 + cneg (state at t=-1 is exactly zero, so the
        # beta-fold baked into b_u2 must be removed for step 0)
        nc.vector.tensor_add(w_a, u_col[:, 0:4], cneg_sb)

        # filler queue: (step at which the work becomes legal, items)
        # pre-pass chunks depend only on DMAs, so schedule them as early as
        # xpool capacity allows -- they must finish WELL before the scan
        # reaches them (the chunk-boundary STT also takes explicit deps)
        filler = []
        for c in range(1, n_tc):
            filler.append(((c - 1) * 290 + 2, pre_pass_items(c)))
        for t_i in range(n_pc - 1):
            filler.append(((t_i + 1) * pcw + 2, post_pass_items(t_i)))
        filler.sort(key=lambda x: x[0])

        last_tile_box = [None]
        gp_insts = [None] * t_len

        # HAM warming: keep the PE busy through the ring's idle window so the
        # clock gate stays at 8/8 (idle scan duty ~55% re-throttles to 1.2GHz)
        dmv = singles.tile([128, 512], bf16)
        nc.vector.memset(dmv, 0.0)
        dummy_ps = psum_d.tile([128, 512], f32)

        def scan_step(jj):
            even = jj % 2 == 0
            cur, nxt = (w_a, w_b) if even else (w_b, w_a)
            rb = rb_a if even else rb_b
            wsq = wsq_a if even else wsq_b
            # squares for the variance (DVE, feeds the stats matmuls)
            nc.vector.tensor_mul(wsq, cur, cur)
            # early matvec tiles run while DVE computes wsq
            wp = psum_w.tile([128, 4], f32)
            pre_last = None
            for (ki, m) in TILE_ORDER[:N_PRE_TILES]:
                pre_last = nc.tensor.matmul(
                    wp[:, m:m + 1], tt_sbs[TILE_ORDER.index((ki, m))],
                    cur[:, ki:ki + 1], start=(ki == 0), stop=(ki == m),
                )
            # stats: Sum_p wsq/S broadcast to all partitions, accumulated over
            # the 4 column chunks into a single PSUM column
            sp = psum_s.tile([128, 1], f32)
            st_first = None
            st_last = None
            for kk in range(4):
                mm = nc.tensor.matmul(
                    sp, ones_sb, wsq[:, kk:kk + 1], start=(kk == 0), stop=(kk == 3),
                    skip_group_check=True,
                )
                if kk == 0:
                    st_first = mm
                st_last = mm
            add_dep_helper(st_first.ins, pre_last.ins, sync=False,
                           reason="stats after early tiles")
            # remaining matvec tiles run while ACT computes rr
            post_first = None
            for (ki, m) in TILE_ORDER[N_PRE_TILES:]:
                mm = nc.tensor.matmul(
                    wp[:, m:m + 1], tt_sbs[TILE_ORDER.index((ki, m))],
                    cur[:, ki:ki + 1], start=(ki == 0), stop=(ki == m),
                )
                if post_first is None:
                    post_first = mm
                    add_dep_helper(post_first.ins, st_last.ins, sync=False,
                                   reason="late tiles after stats")
                last_tile_box[0] = mm
            # rr = rsqrt(var + eps) straight from PSUM (1/S is in the ones)
            nc.scalar.activation(
                out=rb, in_=sp, func=AF.Abs_reciprocal_sqrt,
                bias=eps_sb, scale=1.0,
            )
            # whitened state tw = rr*w (GPSIMD, off the critical ring)
            gp = nc.gpsimd.tensor_scalar(
                out=st_view[:, jj, :], in0=cur, scalar1=rb,
                scalar2=1.0, op0=ALU.mult, op1=ALU.mult,
            )
            gp_insts[jj] = gp
            # serial tail: w_{k+1} = rr*W + uc[k+1]
            stt = nc.vector.scalar_tensor_tensor(
                out=nxt, in0=wp, scalar=rb, in1=u_view[:, jj + 1, :],
                op0=ALU.mult, op1=ALU.add,
            )
            # the STT that first consumes a pre-pass chunk must wait for all
            # of that chunk's evacuations (the strided-slice RAW dep is not
            # reliably auto-tracked)
            if (jj + 1) % PRE_CHUNK == 0 and (jj + 1) // PRE_CHUNK in evac_insts:
                evs = evac_insts[(jj + 1) // PRE_CHUNK]
                assert len(evs) == 8, (
                    f"pre-pass chunk {(jj + 1) // PRE_CHUNK} only has "
                    f"{len(evs)}/8 evacuations emitted by step {jj}"
                )
                for ev in evs:
                    add_dep_helper(stt.ins, ev.ins, sync=True,
                                   reason="scan waits for pre-pass chunk")

        fill_idx = 0
        cur_items = []
        for jj in range(t_len):
            scan_step(jj)
            did_fill = False
            if not cur_items and fill_idx < len(filler) and jj >= filler[fill_idx][0]:
                cur_items = list(filler[fill_idx][1])
                fill_idx += 1
            if cur_items and jj % FILLER_EVERY == 0:
                inst = cur_items.pop(0)()
                did_fill = True
                if inst is not None and last_tile_box[0] is not None:
                    iobj = inst.ins if hasattr(inst, "ins") else inst
                    add_dep_helper(iobj, last_tile_box[0].ins, sync=False,
                                   reason="filler after scan tiles")
            if not did_fill:
                for _ in range(2):
                    dmm = nc.tensor.matmul(dummy_ps, tt_sbs[0], dmv,
                                           start=True, stop=True)
                    add_dep_helper(dmm.ins, last_tile_box[0].ins, sync=False,
                                   reason="warming mm after scan tiles")

        # leftover filler (tail post-pass chunks) runs after the scan
        while cur_items or fill_idx < len(filler):
            if not cur_items and fill_idx < len(filler):
                cur_items = list(filler[fill_idx][1])
                fill_idx += 1
            if cur_items:
                cur_items.pop(0)()
        for item in post_pass_items(n_pc - 1):
            item()

    nc.compile()
    return nc


def _fix_boundaries(Tm, Q, bounds=(128, 256, 384)):
    """Thread 1x1 Schur blocks to the tile boundaries so no 2x2 block
    straddles a multiple of 128 (dtrexc keeps the similarity orthogonal)."""
    from scipy.linalg import lapack

    n = Tm.shape[0]

    def block_starts():
        starts, i = [], 0
        while i < n:
            if i + 1 < n and abs(Tm[i + 1, i]) > 1e-12:
                starts.append((i, 2)); i += 2
            else:
                starts.append((i, 1)); i += 1
        return starts

    for b in bounds:
        tries = 0
        banned = set()
        while abs(Tm[b, b - 1]) > 1e-12 and tries < 64:
            tries += 1
            ones = [p for p, sz in block_starts() if sz == 1 and p not in banned]
            if not ones:
                raise RuntimeError("no usable 1x1 Schur blocks")
            p = min(ones, key=lambda q: abs(q - b))
            if p > b:
                ifst, ilst = p + 1, b + 1
            else:
                ifst, ilst = p + 1, b
            Tm2, Q2, info = lapack.dtrexc(Tm, Q, ifst, ilst)
            if info != 0:
                banned.add(p)
                continue
            Tm, Q = Tm2, Q2
        if abs(Tm[b, b - 1]) > 1e-12:
            raise RuntimeError(f"could not clear Schur 2x2 straddle at {b}")
    return Tm, Q


def host_prep(inputs, t_len=T):
    """Fold parameters on the host; returns (shared dict, per-core xt list)."""
    from ml_dtypes import bfloat16
    import scipy.linalg as sla

    et = np.asarray(inputs["embedded_tokens"], np.float32)
    W_e2s = np.asarray(inputs["W_e2s"], np.float64)
    b_e2s = np.asarray(inputs["b_e2s"], np.float64)
    A = np.asarray(inputs["A"], np.float64)
    Bm = np.asarray(inputs["Bm"], np.float64)
    C = np.asarray(inputs["C"], np.float64)
    gamma = np.asarray(inputs["ln_gamma"], np.float64)
    beta = np.asarray(inputs["ln_beta"], np.float64)
    W_s2o = np.asarray(inputs["W_s2o"], np.float64)
    b_s2o = np.asarray(inputs["b_s2o"], np.float64)

    G = gamma[:, None] * A
    Gt = G - np.outer(G @ np.ones(S) / S, np.ones(S))   # zero row-sums
    Tm, Q = sla.schur(Gt, output="real")
    Tm, Q = _fix_boundaries(Tm, Q)
    for ki in range(4):
        for kj in range(4):
            if ki > kj:
                Tm[128 * ki:128 * ki + 128, 128 * kj:128 * kj + 128] = 0.0
    tt_tiles = np.stack([
        Tm[128 * ki:128 * ki + 128, 128 * m:128 * m + 128]
        for (ki, m) in TILE_ORDER
    ])

    CS = np.eye(S) - np.ones((S, S)) / S                 # centering matrix
    W_u2 = (W_e2s @ Bm) @ CS @ Q                         # [E, S]
    b_u2 = ((b_e2s @ Bm + beta @ A) @ CS) @ Q            # [S]
    cneg = -(((beta @ A) @ CS) @ Q)                      # step-0 fix
    W_O = Q.T @ (gamma[:, None] * C) @ W_s2o             # [S, E]
    b_out = beta @ C @ W_s2o + b_s2o                     # [E]

    wu_tiles = np.ascontiguousarray(
        W_u2.astype(np.float32).reshape(8, 128, 4, 128).transpose(0, 2, 1, 3)
    )  # [k, m, 128, 128]

    shared = {
        "wu": wu_tiles,
        "tt": np.ascontiguousarray(tt_tiles.astype(bfloat16)),
        "wo": np.ascontiguousarray(W_O.astype(bfloat16)),
        "buc": np.ascontiguousarray(b_u2.astype(np.float32).reshape(4, 128).T),
        # bias seeded through the 1/S-ones matmul: sum_p (1/S)*(4*b_out) = b_out
        "bo4": np.ascontiguousarray((4.0 * b_out).astype(bfloat16).reshape(1, E)),
        "cneg": np.ascontiguousarray(cneg.astype(np.float32).reshape(4, 128).T),
        "ones": np.full((128, 128), 1.0 / S, bfloat16),
    }
    xts = [
        np.ascontiguousarray(et[b, :t_len, :].T.astype(np.float32))
        for b in range(et.shape[0])
    ]
    return shared, xts


def kernel(**inputs):
    key = ("nc", T)
    if key not in _CACHE:
        _CACHE[key] = build(T)
    nc = _CACHE[key]

    from concourse.bass_utils import run_bass_kernel_spmd

    shared, xts = host_prep(inputs)
    in_maps = [dict(shared, xt=xts[b]) for b in range(B)]
    res = run_bass_kernel_spmd(nc, in_maps, core_ids=list(range(NCORES)))
    out = np.stack([np.asarray(r["y"], np.float32) for r in res.results], axis=0)
    return out


# revision 13
# speedup vs baseline: 1.6233x; 1.0034x over previous
"""Trainium2 Bass kernel for the LayerNorm-RNN attention variant.

Math (per batch element b, reference semantics):
    u_t   = (x_t @ W_e2s + b_e2s) @ Bm
    y_t   = s_{t-1} @ A + u_t
    s_t   = LN(y_t) * gamma + beta
    out_t = (s_t @ C) @ W_s2o + b_s2o

Key reformulation (all folds exact, done host-side in fp64):
  1. Mean-free weights: G = diag(gamma) @ A,  Gt = G - (G@1/S) 1^T has zero
     row-sums, so W = zc @ Gt is exactly zero-mean for any zc. Tracking the
     CENTERED pre-norm state zc kills the per-step mean/bias bookkeeping:
         zc_{t+1} = rr_t * (zc_t @ Gt) + uc_{t+1},   rr_t = rsqrt(|zc_t|^2/S + eps)
     with uc = centered input injection (centering matrix folded into W_u).
  2. Orthogonal Schur basis: Gt = Q T Q^T (real Schur, 2x2 blocks nudged off
     the 128-boundaries with dtrexc). w = zc @ Q keeps |w| = |zc| (stats
     unchanged) while T is block-upper-triangular: the per-step matvec needs
     only 10 of 16 [128,128] tiles.
  3. Whitened states tw_t = rr_t * w_t are accumulated and folded through
     W_O = Q^T diag(gamma) C W_s2o in a bulk post-pass.

Per-step engine schedule (the serial ring is the wall clock; everything else
hides inside it):
    DVE:  wsq = w*w
    PE :  3 early T tiles | stats: Sum_p wsq/S via 4 accumulating matmuls with
          a 1/S stationary (broadcast to all partitions) | 7 late tiles
    ACT:  rr = rsqrt(var + eps)  directly from PSUM
    DVE:  w' = rr * W + uc_next    (single scalar_tensor_tensor)
    GPSIMD: tw = rr * w            (off the critical ring)

The scan is fully unrolled in Python (no hardware loop); the input pre-pass
(x @ W_u2 chunks) and output post-pass (states @ W_O chunks) are sprinkled
into the PE/ACT idle windows of the scan so they cost ~no wall clock.

Sharding: data-parallel over batch, 1 batch element per NeuronCore (8 cores).
Layouts on chip are column-form: S=512 lives as [128 partitions x 4 free].
"""

import sys
import os
from contextlib import ExitStack

import numpy as np

for _p in ("/opt/trn_rl_repo",):
    if _p not in sys.path and os.path.isdir(_p):
        sys.path.insert(0, _p)

B, T, E, S = 8, 2048, 1024, 512
LN_EPS = 1e-5
NCORES = 8

# block-upper-triangular tile order (ki = contraction chunk, m = output chunk)
TILE_ORDER = [(ki, m) for m in range(4) for ki in range(m + 1)]
N_PRE_TILES = 3     # tiles issued before the stats matmuls
PRE_CHUNK = 512     # pre-pass t-chunk width
POST_CHUNK = 128    # post-pass t-chunk width
FILLER_EVERY = 3    # emit one filler work item every this many scan steps

_CACHE = {}


def build(t_len=T):
    """Build the single-core Bass program (SPMD across 8 cores)."""
    import concourse.bass as bass
    import concourse.bacc as bacc
    from concourse import mybir
    from concourse.tile import TileContext
    from concourse.tile_rust import add_dep_helper

    f32 = mybir.dt.float32
    bf16 = mybir.dt.bfloat16
    AF = mybir.ActivationFunctionType
    ALU = mybir.AluOpType

    n_tc = (t_len + PRE_CHUNK - 1) // PRE_CHUNK
    tcw = min(PRE_CHUNK, t_len)
    pcw = min(POST_CHUNK, t_len)
    n_pc = (t_len + pcw - 1) // pcw

    nc = bacc.Bacc(trn_type="TRN2")

    xt = nc.dram_tensor("xt", [E, t_len], f32, kind="ExternalInput")
    wu = nc.dram_tensor("wu", [8, 4, 128, 128], f32, kind="ExternalInput")
    tt = nc.dram_tensor("tt", [len(TILE_ORDER), 128, 128], bf16, kind="ExternalInput")
    wo = nc.dram_tensor("wo", [S, E], bf16, kind="ExternalInput")
    bud = nc.dram_tensor("buc", [128, 4], f32, kind="ExternalInput")
    bo4d = nc.dram_tensor("bo4", [1, E], bf16, kind="ExternalInput")
    cnegd = nc.dram_tensor("cneg", [128, 4], f32, kind="ExternalInput")
    onesd = nc.dram_tensor("ones", [128, 128], bf16, kind="ExternalInput")
    y = nc.dram_tensor("y", [t_len, E], f32, kind="ExternalOutput")

    with ExitStack() as ctx:
        tc = ctx.enter_context(TileContext(nc))
        singles = ctx.enter_context(tc.tile_pool(name="singles", bufs=1))
        xpool = ctx.enter_context(tc.tile_pool(name="xpool", bufs=16))
        psum_big = ctx.enter_context(tc.tile_pool(name="psum_big", bufs=2, space="PSUM"))
        psum_w = ctx.enter_context(tc.tile_pool(name="psum_w", bufs=2, space="PSUM"))
        psum_s = ctx.enter_context(tc.tile_pool(name="psum_s", bufs=1, space="PSUM"))
        opool = ctx.enter_context(tc.tile_pool(name="opool", bufs=2))

        # ---- resident weights / constants ----
        wu_sb = singles.tile([128, 8, 4, 128], f32)
        nc.sync.dma_start(out=wu_sb, in_=wu.rearrange("k m p q -> p k m q"))
        tt_sbs = []
        for i in range(len(TILE_ORDER)):
            t_sb = singles.tile([128, 128], bf16, tag=f"tt{i}")
            nc.sync.dma_start(out=t_sb, in_=tt[i])
            tt_sbs.append(t_sb)
        wo_sb = singles.tile([128, 4, E], bf16)
        nc.sync.dma_start(out=wo_sb, in_=wo.rearrange("(k p) e -> p k e", p=128))
        bu_sb = singles.tile([128, 4], f32)
        nc.sync.dma_start(out=bu_sb, in_=bud[:])
        ones_sb = singles.tile([128, 128], bf16)
        nc.sync.dma_start(out=ones_sb, in_=onesd[:])
        bo4_ap = bo4d[:]
        bo4_sb = singles.tile([128, E], bf16)
        nc.sync.dma_start(
            out=bo4_sb,
            in_=bass.AP(tensor=bo4_ap.tensor, offset=bo4_ap.offset, ap=[[0, 128], [1, E]]),
        )
        cneg_sb = singles.tile([128, 4], f32)
        nc.sync.dma_start(out=cneg_sb, in_=cnegd[:])
        eps_sb = singles.tile([128, 1], f32)
        nc.vector.memset(eps_sb, LN_EPS)

        u_col = singles.tile([128, (t_len + 1) * 4], f32)
        states = singles.tile([128, t_len * 4], bf16)
        u_view = u_col.rearrange("p (t f) -> p t f", f=4)
        st_view = states.rearrange("p (t f) -> p t f", f=4)
        nc.vector.memset(u_col[:, t_len * 4:(t_len + 1) * 4], 0.0)

        # ---- pre-pass emitter: uc[t-chunk] = (x @ W_u2).T + b_u2, col form ----
        evac_insts = {}   # (chunk, half) -> list of evacuation ACT instructions

        def pre_pass_items(c, halves=(0, 1)):
            """Return filler callables computing u_col halves of t-chunk c."""
            xts = [None] * 8
            items = []

            def load_x():
                for e in range(8):
                    xts[e] = xpool.tile([128, tcw], f32, tag="xt", name="xtile")
                    nc.sync.dma_start(
                        out=xts[e],
                        in_=xt[e * 128:(e + 1) * 128, c * tcw:(c + 1) * tcw],
                    )
                return None
            items.append(load_x)
            for h in halves:
                evac_insts[(c, h)] = []
                for m in range(4):
                    ps_box = [None]

                    def mk_mm(m=m, k=0, h=h, ps_box=ps_box):
                        def mm():
                            if ps_box[0] is None:
                                ps_box[0] = psum_big.tile([128, 256], f32, tag="pre", name="pre_ps")
                            return nc.tensor.matmul(
                                ps_box[0],
                                wu_sb[:, k, m, :],
                                xts[k][:, h * 256:(h + 1) * 256],
                                start=(k == 0), stop=(k == 7),
                            )
                        return mm

                    def mk_evac(m=m, h=h, q=0, ps_box=ps_box):
                        def evac():
                            lo = c * tcw + h * 256 + q * 128
                            inst = nc.scalar.activation(
                                out=u_view[:, lo:lo + 128, m],
                                in_=ps_box[0][:, q * 128:(q + 1) * 128],
                                func=AF.Identity, bias=bu_sb[:, m:m + 1], scale=1.0,
                            )
                            evac_insts[(c, h)].append(inst)
                            return inst
                        return evac
                    for k in range(8):
                        items.append(mk_mm(m=m, k=k, h=h, ps_box=ps_box))
                    for q in range(2):
                        items.append(mk_evac(m=m, h=h, q=q, ps_box=ps_box))
            return items

        # ---- post-pass emitter: y[t-chunk] = states @ W_O + b_out ----
        def post_pass_items(t_i):
            ob_box = [None]
            items = []

            def mk_mm(ec=0, h=0, kk=0, ps_box=None):
                def mm():
                    if ob_box[0] is None:
                        ob_box[0] = opool.tile([128, E], f32, name="ob")
                    if ps_box[0] is None:
                        ps_box[0] = psum_big.tile([128, 256], f32, tag="post", name="post_ps")
                    lo = ec * 512 + h * 256
                    if kk < 0:   # bias seed: (1/S ones)^T @ (4*b_out) = b_out
                        inst = nc.tensor.matmul(
                            ps_box[0], ones_sb, bo4_sb[:, lo:lo + 256],
                            start=True, stop=False,
                        )
                        guard = gp_insts[min((t_i + 1) * pcw, t_len) - 1]
                        if guard is not None:
                            add_dep_helper(inst.ins, guard.ins, sync=True,
                                           reason="post-pass waits for states chunk")
                        return inst
                    return nc.tensor.matmul(
                        ps_box[0],
                        st_view[:, t_i * pcw:(t_i + 1) * pcw, kk],
                        wo_sb[:, kk, lo:lo + 256],
                        start=False, stop=(kk == 3),
                    )
                return mm

            def mk_copy(ec=0, h=0, q=0, ps_box=None):
                def cp():
                    lo = ec * 512 + h * 256 + q * 128
                    return nc.scalar.activation(
                        out=ob_box[0][:pcw, lo:lo + 128],
                        in_=ps_box[0][:pcw, q * 128:(q + 1) * 128],
                        func=AF.Identity, scale=1.0,
                    )
                return cp

            for ec in range(2):
                for h in range(2):
                    ps_box = [None]
                    for kk in (-1, 0, 1, 2, 3):
                        items.append(mk_mm(ec=ec, h=h, kk=kk, ps_box=ps_box))
                    for q in range(2):
                        items.append(mk_copy(ec=ec, h=h, q=q, ps_box=ps_box))

            def store():
                nc.sync.dma_start(
                    out=y[t_i * pcw:(t_i + 1) * pcw, :], in_=ob_box[0][:pcw, :]
                )
                return None
            items.append(store)
            return items

        # ---- scan state ----
        w_a = singles.tile([128, 4], bf16)
        w_b = singles.tile([128, 4], bf16)
        wsq_a = singles.tile([128, 4], bf16)
        wsq_b = singles.tile([128, 4], bf16)
        rb_a = singles.tile([128, 1], f32)
        rb_b = singles.tile([128, 1], f32)

        # first half of pre-pass chunk 0 runs up front (the scan needs it
        # immediately); the second half is the first filler in the queue
        for item in pre_pass_items(0, halves=(0,)):
            item()

        # prologue: w_0 = uc_0 + cneg (state at t=-1 is exactly zero, so the
        # beta-fold baked into b_u2 must be removed for step 0)
        nc.vector.tensor_add(w_a, u_col[:, 0:4], cneg_sb)

        # filler queue: (step at which the work becomes legal, items)
        # pre-pass chunks depend only on DMAs, so schedule them as early as
        # xpool capacity allows -- they must finish WELL before the scan
        # reaches them (the chunk-boundary STT also takes explicit deps)
        filler = [(0, pre_pass_items(0, halves=(1,)))]
        for c in range(1, n_tc):
            filler.append(((c - 1) * 220 + 40, pre_pass_items(c)))
        for t_i in range(n_pc - 1):
            filler.append(((t_i + 1) * pcw + 2, post_pass_items(t_i)))
        filler.sort(key=lambda x: x[0])

        last_tile_box = [None]
        gp_insts = [None] * t_len

        def scan_step(jj):
            even = jj % 2 == 0
            cur, nxt = (w_a, w_b) if even else (w_b, w_a)
            rb = rb_a if even else rb_b
            wsq = wsq_a if even else wsq_b
            # squares for the variance (DVE, feeds the stats matmuls)
            nc.vector.tensor_mul(wsq, cur, cur)
            # early matvec tiles run while DVE computes wsq
            wp = psum_w.tile([128, 4], f32)
            pre_last = None
            for (ki, m) in TILE_ORDER[:N_PRE_TILES]:
                pre_last = nc.tensor.matmul(
                    wp[:, m:m + 1], tt_sbs[TILE_ORDER.index((ki, m))],
                    cur[:, ki:ki + 1], start=(ki == 0), stop=(ki == m),
                )
            # stats: Sum_p wsq/S broadcast to all partitions, accumulated over
            # the 4 column chunks into a single PSUM column
            sp = psum_s.tile([128, 1], f32)
            st_first = None
            st_last = None
            for kk in range(4):
                mm = nc.tensor.matmul(
                    sp, ones_sb, wsq[:, kk:kk + 1], start=(kk == 0), stop=(kk == 3),
                    skip_group_check=True,
                )
                if kk == 0:
                    st_first = mm
                st_last = mm
            add_dep_helper(st_first.ins, pre_last.ins, sync=False,
                           reason="stats after early tiles")
            # remaining matvec tiles run while ACT computes rr
            post_first = None
            for (ki, m) in TILE_ORDER[N_PRE_TILES:]:
                mm = nc.tensor.matmul(
                    wp[:, m:m + 1], tt_sbs[TILE_ORDER.index((ki, m))],
                    cur[:, ki:ki + 1], start=(ki == 0), stop=(ki == m),
                )
                if post_first is None:
                    post_first = mm
                    add_dep_helper(post_first.ins, st_last.ins, sync=False,
                                   reason="late tiles after stats")
                last_tile_box[0] = mm
            # rr = rsqrt(var + eps) straight from PSUM (1/S is in the ones)
            nc.scalar.activation(
                out=rb, in_=sp, func=AF.Abs_reciprocal_sqrt,
                bias=eps_sb, scale=1.0,
            )
            # whitened state tw = rr*w (GPSIMD, off the critical ring)
            gp = nc.gpsimd.tensor_scalar(
                out=st_view[:, jj, :], in0=cur, scalar1=rb,
                scalar2=1.0, op0=ALU.mult, op1=ALU.mult,
            )
            gp_insts[jj] = gp
            # serial tail: w_{k+1} = rr*W + uc[k+1]
            stt = nc.vector.scalar_tensor_tensor(
                out=nxt, in0=wp, scalar=rb, in1=u_view[:, jj + 1, :],
                op0=ALU.mult, op1=ALU.add,
            )
            # the STT that first consumes a pre-pass chunk must wait for all
            # of that chunk's evacuations (the strided-slice RAW dep is not
            # reliably auto-tracked)
            if (jj + 1) % 256 == 0:
                key = ((jj + 1) // PRE_CHUNK, ((jj + 1) // 256) % 2)
                if key in evac_insts:
                    evs = evac_insts[key]
                    assert len(evs) == 8, (
                        f"pre-pass half-chunk {key} only has "
                        f"{len(evs)}/8 evacuations emitted by step {jj}"
                    )
                    for ev in evs:
                        add_dep_helper(stt.ins, ev.ins, sync=True,
                                       reason="scan waits for pre-pass half")

        fill_idx = 0
        cur_items = []
        for jj in range(t_len):
            scan_step(jj)
            if not cur_items and fill_idx < len(filler) and jj >= filler[fill_idx][0]:
                cur_items = list(filler[fill_idx][1])
                fill_idx += 1
            if cur_items and jj % FILLER_EVERY == 0:
                inst = cur_items.pop(0)()
                if inst is not None and last_tile_box[0] is not None:
                    iobj = inst.ins if hasattr(inst, "ins") else inst
                    add_dep_helper(iobj, last_tile_box[0].ins, sync=False,
                                   reason="filler after scan tiles")
        last_tile_box = [None]
        gp_insts = [None] * t_len

        # leftover filler (tail post-pass chunks) runs after the scan
        while cur_items or fill_idx < len(filler):
            if not cur_items and fill_idx < len(filler):
                cur_items = list(filler[fill_idx][1])
                fill_idx += 1
            if cur_items:
                cur_items.pop(0)()
        for item in post_pass_items(n_pc - 1):
            item()

    nc.compile()
    return nc


def _fix_boundaries(Tm, Q, bounds=(128, 256, 384)):
    """Thread 1x1 Schur blocks to the tile boundaries so no 2x2 block
    straddles a multiple of 128 (dtrexc keeps the similarity orthogonal)."""
    from scipy.linalg import lapack

    n = Tm.shape[0]

    def block_starts():
        starts, i = [], 0
        while i < n:
            if i + 1 < n and abs(Tm[i + 1, i]) > 1e-12:
                starts.append((i, 2)); i += 2
            else:
                starts.append((i, 1)); i += 1
        return starts

    for b in bounds:
        tries = 0
        banned = set()
        while abs(Tm[b, b - 1]) > 1e-12 and tries < 64:
            tries += 1
            ones = [p for p, sz in block_starts() if sz == 1 and p not in banned]
            if not ones:
                raise RuntimeError("no usable 1x1 Schur blocks")
            p = min(ones, key=lambda q: abs(q - b))
            if p > b:
                ifst, ilst = p + 1, b + 1
            else:
                ifst, ilst = p + 1, b
            Tm2, Q2, info = lapack.dtrexc(Tm, Q, ifst, ilst)
            if info != 0:
                banned.add(p)
                continue
            Tm, Q = Tm2, Q2
        if abs(Tm[b, b - 1]) > 1e-12:
            raise RuntimeError(f"could not clear Schur 2x2 straddle at {b}")
    return Tm, Q


def host_prep(inputs, t_len=T):
    """Fold parameters on the host; returns (shared dict, per-core xt list)."""
    from ml_dtypes import bfloat16
    import scipy.linalg as sla

    et = np.asarray(inputs["embedded_tokens"], np.float32)
    W_e2s = np.asarray(inputs["W_e2s"], np.float64)
    b_e2s = np.asarray(inputs["b_e2s"], np.float64)
    A = np.asarray(inputs["A"], np.float64)
    Bm = np.asarray(inputs["Bm"], np.float64)
    C = np.asarray(inputs["C"], np.float64)
    gamma = np.asarray(inputs["ln_gamma"], np.float64)
    beta = np.asarray(inputs["ln_beta"], np.float64)
    W_s2o = np.asarray(inputs["W_s2o"], np.float64)
    b_s2o = np.asarray(inputs["b_s2o"], np.float64)

    G = gamma[:, None] * A
    Gt = G - np.outer(G @ np.ones(S) / S, np.ones(S))   # zero row-sums
    Tm, Q = sla.schur(Gt, output="real")
    Tm, Q = _fix_boundaries(Tm, Q)
    for ki in range(4):
        for kj in range(4):
            if ki > kj:
                Tm[128 * ki:128 * ki + 128, 128 * kj:128 * kj + 128] = 0.0
    tt_tiles = np.stack([
        Tm[128 * ki:128 * ki + 128, 128 * m:128 * m + 128]
        for (ki, m) in TILE_ORDER
    ])

    CS = np.eye(S) - np.ones((S, S)) / S                 # centering matrix
    W_u2 = (W_e2s @ Bm) @ CS @ Q                         # [E, S]
    b_u2 = ((b_e2s @ Bm + beta @ A) @ CS) @ Q            # [S]
    cneg = -(((beta @ A) @ CS) @ Q)                      # step-0 fix
    W_O = Q.T @ (gamma[:, None] * C) @ W_s2o             # [S, E]
    b_out = beta @ C @ W_s2o + b_s2o                     # [E]

    wu_tiles = np.ascontiguousarray(
        W_u2.astype(np.float32).reshape(8, 128, 4, 128).transpose(0, 2, 1, 3)
    )  # [k, m, 128, 128]

    shared = {
        "wu": wu_tiles,
        "tt": np.ascontiguousarray(tt_tiles.astype(bfloat16)),
        "wo": np.ascontiguousarray(W_O.astype(bfloat16)),
        "buc": np.ascontiguousarray(b_u2.astype(np.float32).reshape(4, 128).T),
        # bias seeded through the 1/S-ones matmul: sum_p (1/S)*(4*b_out) = b_out
        "bo4": np.ascontiguousarray((4.0 * b_out).astype(bfloat16).reshape(1, E)),
        "cneg": np.ascontiguousarray(cneg.astype(np.float32).reshape(4, 128).T),
        "ones": np.full((128, 128), 1.0 / S, bfloat16),
    }
    xts = [
        np.ascontiguousarray(et[b, :t_len, :].T.astype(np.float32))
        for b in range(et.shape[0])
    ]
    return shared, xts


def kernel(**inputs):
    key = ("nc", T)
    if key not in _CACHE:
        _CACHE[key] = build(T)
    nc = _CACHE[key]

    from concourse.bass_utils import run_bass_kernel_spmd

    shared, xts = host_prep(inputs)
    in_maps = [dict(shared, xt=xts[b]) for b in range(B)]
    res = run_bass_kernel_spmd(nc, in_maps, core_ids=list(range(NCORES)))
    out = np.stack([np.asarray(r["y"], np.float32) for r in res.results], axis=0)
    return out
